# revision 35
# baseline (speedup 1.0000x reference)
"""MatchingNet model kernel for 8 Trainium2 NeuronCores (v2, fp8 exchange).

Computation (reference semantics, N=4096, E=512, G=256, V=50000, R=1000):
  x  = embedding[input]          (N, E)
  ex = embedding[set_inputs]     (2, N, E)
  g_out = bidirectional 2-step LSTM over ex   (2, N, E)
  fh = lstm_f(x) + x             (N, E)          [single step, zero state]
  scores[b] = g_out[b] @ fh.T    (2, N, N)
  a = softmax(scores, axis=0)    -- softmax over b (size 2), pointwise in (n,m)
  r[b] = a[b] @ g_out[b]         (2, N, E)
  dot/nr/ng reductions over n -> cos (2, E) -> tiny tail -> softmax (R,)

Sharding: data-parallel over N. Core k owns rows [512k, 512k+512).
a[0] = sigmoid(D), a[1] = 1 - a[0] with D = (g0 - g1) @ fh.T.
r0 = A0 @ g0.  r1 = s1 - q with q = A0 @ g1 and s1[e] = sum_m g1[m,e];
the s1-dependent parts of dot1/sr1 are reconstructed on the host from
per-core partials (t1 = local colsum of g1, sum_q, sq_q, dot1q), so the
device never materializes a[1].

Exchange (all fp8 e4m3, partition-major blocks of 2KB rows):
  AG1: fh.T   (256KB/core -> 2MB)   fired right after the f-LSTM
  AG2a: g0    (256KB/core -> 2MB)   fired right after g0's transposes
  AG2b: g1    (256KB/core -> 2MB)
D1/D2 matmuls run in fp8 DoubleRow mode (2 k-subtiles per call).
"""

import os
import sys

import numpy as np

for _p in ("/opt/trn_rl_repo", os.path.expanduser("~/.axon_site/_ro/trn_rl_repo")):
    if os.path.isdir(_p) and _p not in sys.path:
        sys.path.insert(0, _p)

import concourse.bacc as bacc
import concourse.bass as bass
import concourse.mybir as mybir
import concourse.tile as tile
from concourse import bass_utils
from concourse.masks import make_identity

N, E, G, V, R = 4096, 512, 256, 50000, 1000
NCORES = 8
NL = N // NCORES  # 512 rows per core
P = 128
NE = E // P   # 4 e-chunks
NH = G // P   # 2 hidden chunks for the g-LSTM
NMB = N // P  # 32 global m-blocks
EPS = 1e-8

F32 = mybir.dt.float32
BF16 = mybir.dt.bfloat16
FP8 = mybir.dt.float8e4
I32 = mybir.dt.int32
AF = mybir.ActivationFunctionType
ALU = mybir.AluOpType
DR = mybir.MatmulPerfMode.DoubleRow


def _lstm_cell(nc, pools, H, xT, W_sb, U_sb, hprevT, cprevT, bias_sb, h_out,
               c_out, gates=(0, 1, 2, 3), packed=(0, 1, 2, 3)):
    """Emit one LSTM cell, transposed layout (feature on partition, n free).

    W_sb: (P, NE, len(packed)*H) packed in `packed` gate order; U_sb likewise
    or None.  bias_sb: (P, len(packed)*H//P).  h_out/c_out: (P, H//P, NL).
    Zero-state cells pass gates without 1 (forget) and cprevT=None.
    """
    pg, gp, tp = pools["pg"], pools["gates"], pools["tmp"]
    hc = H // P
    pos = {g: i for i, g in enumerate(packed)}
    gb = {}
    for g in gates:
        gb[g] = gp.tile([P, hc, NL], F32, tag=f"gate{g}", bufs=1, name=f"gb{g}")
        for s in range(hc):
            jc = pos[g] * hc + s
            ps = pg.tile([P, NL], F32, tag="pg", bufs=4, name="ps_gate")
            js = slice(jc * P, (jc + 1) * P)
            for kt in range(NE):
                nc.tensor.matmul(
                    ps[:], W_sb[:, kt, js], xT[:, kt, :],
                    start=(kt == 0), stop=(U_sb is None and kt == NE - 1))
            if U_sb is not None:
                hcu = hprevT.shape[1]
                for kt in range(hcu):
                    nc.tensor.matmul(
                        ps[:], U_sb[:, kt, js], hprevT[:, kt, :],
                        start=False, stop=(kt == hcu - 1))
            func = AF.Tanh if g == 2 else AF.Sigmoid
            nc.scalar.activation(
                out=gb[g][:, s, :], in_=ps[:], func=func,
                bias=bias_sb[:, jc:jc + 1], scale=1.0)
    for s in range(hc):
        i_, g_, o_ = gb[0][:, s, :], gb[2][:, s, :], gb[3][:, s, :]
        if c_out is None:
            c_s = tp.tile([P, NL], F32, tag="ctmp", bufs=2, name="ctmp")
        else:
            c_s = c_out[:, s, :]
        if cprevT is None:
            nc.vector.tensor_mul(c_s, i_, g_)
        else:
            f_ = gb[1][:, s, :]
            ig = tp.tile([P, NL], F32, tag="ig", bufs=2, name="ig")
            nc.vector.tensor_mul(ig[:], i_, g_)
            nc.vector.tensor_mul(c_s, f_, cprevT[:, s, :])
            nc.vector.tensor_add(c_s, c_s, ig[:])
        tc_ = tp.tile([P, NL], F32, tag="tanhc", bufs=2, name="tanhc")
        nc.scalar.activation(out=tc_[:], in_=c_s, func=AF.Tanh)
        nc.vector.tensor_mul(h_out[:, s, :], o_, tc_[:])


def _gather_T(nc, pools, emb, idx_dram, ident, dstT):
    """Gather NL embedding rows and transpose into dstT (P, NE, NL)."""
    ip, rp, pt, cp = pools["idx"], pools["raw"], pools["pt"], pools["tmp"]
    for t in range(NL // P):
        idx_t = ip.tile([P, 1], I32, tag="idx", bufs=4, name="idx_t")
        nc.sync.dma_start(out=idx_t[:], in_=idx_dram[t * P:(t + 1) * P, :])
        raw = rp.tile([P, E], F32, tag="raw", bufs=4, name="raw")
        nc.gpsimd.indirect_dma_start(
            out=raw[:], out_offset=None, in_=emb[:],
            in_offset=bass.IndirectOffsetOnAxis(ap=idx_t[:, :1], axis=0))
        for et in range(NE):
            ptile = pt.tile([P, P], F32, tag="pt", bufs=2, name="ptile")
            nc.tensor.transpose(
                out=ptile[:], in_=raw[:, et * P:(et + 1) * P], identity=ident[:])
            nc.vector.tensor_copy(
                out=dstT[:, et, t * P:(t + 1) * P], in_=ptile[:])


def build_program():
    nc = bacc.Bacc("TRN2", target_bir_lowering=False, debug=False,
                   enable_asserts=False, num_devices=NCORES)
    dram = lambda name, shape, dt=F32, kind="ExternalInput": \
        nc.dram_tensor(name, shape, dt, kind=kind).ap()

    emb = dram("emb", [V, E])
    idx_x = dram("idx_x", [NL, 1], I32)
    idx_e0 = dram("idx_e0", [NL, 1], I32)
    idx_e1 = dram("idx_e1", [NL, 1], I32)
    # weights pre-laid-out on host as lhsT tiles [p, kt, j]
    wgf = dram("wgf", [P, NE, 4 * G], BF16)
    wgr = dram("wgr", [P, NE, 4 * G], BF16)
    ugf = dram("ugf", [P, NH, 4 * G], BF16)
    ugr = dram("ugr", [P, NH, 4 * G], BF16)
    wf = dram("wf", [P, NE, 3 * E], BF16)   # i, g, o gates only (zero state)
    bgf = dram("bgf", [P, 8])
    bgr = dram("bgr", [P, 8])
    bf = dram("bf", [P, 12])
    out_a = dram("out_a", [P, 24], kind="ExternalOutput")
    out_d = dram("out_d", [P, 8], kind="ExternalOutput")

    with tile.TileContext(nc) as tc:
        _emit(tc, locals())
    nc.compile()
    return nc


def _emit(tc, T):
    nc = tc.nc
    rg = [list(range(NCORES))]
    from contextlib import ExitStack
    ctx = ExitStack()
    with ctx:
        glob = ctx.enter_context(tc.tile_pool(name="glob", bufs=1))
        dramp = ctx.enter_context(tc.tile_pool(name="dramp", bufs=1, space="DRAM"))

        ident = glob.tile([P, P], F32)
        make_identity(nc, ident)
        identb = glob.tile([P, P], BF16)
        nc.vector.tensor_copy(out=identb[:], in_=ident[:])

        # fp8 exchange buffers, partition-major 2KB rows
        ag1_src = dramp.tile([P, NE * NL], FP8)
        ag1_dst = dramp.tile([NCORES * P, NE * NL], FP8, addr_space="Shared")
        ag2a_src = dramp.tile([P, 4 * E], FP8)
        ag2a_dst = dramp.tile([NCORES * P, 4 * E], FP8, addr_space="Shared")
        ag2b_src = dramp.tile([P, 4 * E], FP8)
        ag2b_dst = dramp.tile([NCORES * P, 4 * E], FP8, addr_space="Shared")

        # long-lived activations
        g0T = glob.tile([P, NE, NL], BF16)
        g1T = glob.tile([P, NE, NL], BF16)
        dg8 = glob.tile([P, NE, NL], FP8)
        A0T = glob.tile([P, NMB, NL], FP8)
        out_act = glob.tile([P, 24], F32)
        out_dve = glob.tile([P, 8], F32)

        with tc.tile_pool(name="wpool", bufs=1) as wp, \
             tc.tile_pool(name="acts", bufs=1) as ap_, \
             tc.tile_pool(name="gates", bufs=1) as gp, \
             tc.tile_pool(name="tmp", bufs=1) as tp, \
             tc.tile_pool(name="idx", bufs=1) as ip, \
             tc.tile_pool(name="raw", bufs=1) as rp, \
             tc.tile_pool(name="fhk", bufs=1) as fkp, \
             tc.tile_pool(name="tps", bufs=1) as tsp:
            lstm_psum = tc.tile_pool(name="pg", bufs=1, space="PSUM")
            pgp = lstm_psum.__enter__()
            ptp_cm = tc.tile_pool(name="pt", bufs=1, space="PSUM")
            ptp = ptp_cm.__enter__()
            pools = {"pg": pgp, "gates": gp, "tmp": tp, "idx": ip,
                     "raw": rp, "pt": ptp}

            # ---- gathers: e0 first (g-LSTM start), then x (f-LSTM has
            # slack until the barrier ends), then e1 ----
            xT = ap_.tile([P, NE, NL], BF16)
            e0T = ap_.tile([P, NE, NL], BF16)
            e1T = ap_.tile([P, NE, NL], BF16)
            _gather_T(nc, pools, T["emb"], T["idx_e0"], ident, e0T)
            _gather_T(nc, pools, T["emb"], T["idx_x"], ident, xT)
            _gather_T(nc, pools, T["emb"], T["idx_e1"], ident, e1T)

            # weight DMA order follows first use: biases, g-LSTM stage-1
            # quarters, f-LSTM, recurrent weights, forget-gate quarters
            w_sb = {}
            for nm in ("bgf", "bgr"):
                w_sb[nm] = wp.tile([P, 8], F32, name=nm + "_sb")
                nc.sync.dma_start(out=w_sb[nm][:], in_=T[nm][:])
            bf_sb = wp.tile([P, 12], F32)
            nc.sync.dma_start(out=bf_sb[:], in_=T["bf"][:])
            for nm, kt in (("wgf", NE), ("wgr", NE), ("ugf", NH), ("ugr", NH)):
                w_sb[nm] = wp.tile([P, kt, 4 * G], BF16, name=nm + "_sb")
            wf_sb = wp.tile([P, NE, 3 * E], BF16)
            for nm in ("wgf", "wgr"):
                for q in (0, 2, 3):
                    qs = slice(q * G, (q + 1) * G)
                    nc.sync.dma_start(out=w_sb[nm][:, :, qs],
                                      in_=T[nm][:, :, qs])
            for q in range(3):   # chunked so the i-gate matmuls start early
                qs = slice(q * E, (q + 1) * E)
                nc.sync.dma_start(out=wf_sb[:, :, qs], in_=T["wf"][:, :, qs])
            for nm in ("ugf", "ugr"):
                nc.sync.dma_start(out=w_sb[nm][:], in_=T[nm][:])
            for nm in ("wgf", "wgr"):   # forget gate: only stage 2 needs it
                qs = slice(1 * G, 2 * G)
                nc.sync.dma_start(out=w_sb[nm][:, :, qs], in_=T[nm][:, :, qs])

            fhT = ap_.tile([P, NE, NL], BF16)
            _lstm_cell(nc, pools, E, xT, wf_sb, None, None, None, bf_sb, fhT,
                       None, gates=(0, 2, 3), packed=(0, 2, 3))
            fh8 = ap_.tile([P, NE, NL], FP8, name="fh8")
            for et in range(NE):
                nc.vector.tensor_add(fhT[:, et, :], fhT[:, et, :], xT[:, et, :])
                nc.vector.tensor_copy(out=fh8[:, et, :], in_=fhT[:, et, :])
            nc.sync.dma_start(
                out=ag1_src[:], in_=fh8[:].rearrange("p et n -> p (et n)"))
            nc.gpsimd.collective_compute(
                "AllGather", ALU.bypass, replica_groups=rg,
                ins=[ag1_src[:].opt()], outs=[ag1_dst[:].opt()])

            # ---- g-LSTM with transposes emitted right after each cell ----
            # g1 = [hf1, hr1] completes before g0 = [hf0, hr0] (rev0 is the
            # last cell), so g1's exchange fires first and the q-phase of D2
            # runs first.  Emitting each transpose batch immediately after
            # its producing cell keeps them ahead of the AG1-gated D1 work
            # in every engine stream.
            s0_t = tsp.tile([P, NE, E], FP8, tag="s0", bufs=1, name="s0")
            s1_t = tsp.tile([P, NE, E], FP8, tag="s1", bufs=1, name="s1")

            def emit_tr(gT, e0_, s_t):
                for nt in range(NL // P):
                    ptile = ptp.tile([P, 2, P], BF16, tag="ptg", bufs=2,
                                     name="ptg")
                    for j in range(2):
                        nc.tensor.transpose(
                            out=ptile[:, j, :],
                            in_=gT[:, e0_ + j, nt * P:(nt + 1) * P],
                            identity=identb[:])
                    nc.vector.tensor_copy(
                        out=s_t[:, nt, e0_ * P:(e0_ + 2) * P], in_=ptile[:])

            cfT = ap_.tile([P, NH, NL], F32, name="cfT")
            crT = ap_.tile([P, NH, NL], F32, name="crT")
            c2T = ap_.tile([P, NH, NL], F32, name="c2T")
            c3T = ap_.tile([P, NH, NL], F32, name="c3T")
            hf0 = g0T[:, 0:NH, :]
            hf1 = g1T[:, 0:NH, :]
            hr1 = g1T[:, NH:NE, :]
            hr0 = g0T[:, NH:NE, :]
            _lstm_cell(nc, pools, G, e0T, w_sb["wgf"], None, None, None,
                       w_sb["bgf"], hf0, cfT, gates=(0, 2, 3))
            emit_tr(g0T, 0, s0_t)
            _lstm_cell(nc, pools, G, e1T, w_sb["wgr"], None, None, None,
                       w_sb["bgr"], hr1, crT, gates=(0, 2, 3))
            emit_tr(g1T, NH, s1_t)
            # forget-gate weight quarters land last; stage 2 needs them
            _lstm_cell(nc, pools, G, e1T, w_sb["wgf"], w_sb["ugf"], hf0, cfT,
                       w_sb["bgf"], hf1, c2T)
            emit_tr(g1T, 0, s1_t)
            with tc.high_priority():
                nc.sync.dma_start(
                    out=ag2b_src[:],
                    in_=s1_t[:].rearrange("p s e -> p (s e)"))
                nc.gpsimd.collective_compute(
                    "AllGather", ALU.bypass, replica_groups=rg,
                    ins=[ag2b_src[:].opt()], outs=[ag2b_dst[:].opt()])
            _lstm_cell(nc, pools, G, e0T, w_sb["wgr"], w_sb["ugr"], hr1, crT,
                       w_sb["bgr"], hr0, c3T)
            emit_tr(g0T, NH, s0_t)
            with tc.high_priority():
                nc.sync.dma_start(
                    out=ag2a_src[:],
                    in_=s0_t[:].rearrange("p s e -> p (s e)"))
                nc.gpsimd.collective_compute(
                    "AllGather", ALU.bypass, replica_groups=rg,
                    ins=[ag2a_src[:].opt()], outs=[ag2a_dst[:].opt()])
            # scheduler fence: nothing downstream (AG1-gated fhk loads, D1
            # matmuls/sigmoids, gpk loads) may be reordered ahead of the
            # transpose casts / ag2 source writes / collective fires above.
            tc.no_sync_barrier()
            for et in range(NE):
                nc.vector.tensor_sub(dg8[:, et, :], g0T[:, et, :],
                                     g1T[:, et, :])
            # sg / t1 reductions now: the Act engine is idle while AG1 runs
            for b, gT in ((0, g0T), (1, g1T)):
                for et in range(NE):
                    scr = tp.tile([P, NL], F32, tag="scr", bufs=2, name="scr")
                    c0 = 12 + 4 * b + et
                    nc.scalar.activation(
                        out=scr[:], in_=gT[:, et, :],
                        func=AF.Square, accum_out=out_act[:, c0:c0 + 1])
            for et in range(NE):
                scr = tp.tile([P, NL], F32, tag="scr", bufs=2, name="scr")
                nc.scalar.activation(
                    out=scr[:], in_=g1T[:, et, :],
                    func=AF.Copy, accum_out=out_act[:, 20 + et:20 + et + 1])
            ptp_cm.__exit__(None, None, None)
            lstm_psum.__exit__(None, None, None)
            pd_cm = tc.tile_pool(name="pd", bufs=1, space="PSUM")
            pdp = pd_cm.__enter__()

            # ---- D1: A0 = sigmoid((g0-g1) @ fh_all.T), fp8 DoubleRow ----
            # Two DMAs stage the 8 gathered fh blocks (halves pipeline D1's
            # start); single waits on the SP queue keep the scheduler from
            # interleaving AG1-gated loads ahead of the ag2 source writes.
            fhk = fkp.tile([P, NCORES, NE, NL], FP8, name="fhk_all")
            for hk in range(2):
                nc.sync.dma_start(
                    out=fhk[:, hk * 4:(hk + 1) * 4, :, :],
                    in_=ag1_dst[hk * 4 * P:(hk + 1) * 4 * P, :].rearrange(
                        "(k p) (et n) -> p k et n", p=P, et=NE))
            for k in range(NCORES):
                for c in range(0, NL // P, 2):
                    mb = k * (NL // P) + c
                    pd = pdp.tile([P, 2, NL], F32, tag="pd", bufs=2, name="pd")
                    for h in range(2):
                        cs = slice((c + h) * P, (c + h + 1) * P)
                        nc.tensor.matmul(
                            pd[:, h, :], fhk[:, k, 0:2, cs], dg8[:, 0:2, :],
                            start=True, stop=False, perf_mode=DR)
                        nc.tensor.matmul(
                            pd[:, h, :], fhk[:, k, 2:4, cs], dg8[:, 2:4, :],
                            start=False, stop=True, perf_mode=DR)
                    nc.scalar.activation(
                        out=A0T[:, mb:mb + 2, :], in_=pd[:], func=AF.Sigmoid)
            pd_cm.__exit__(None, None, None)

        # ---- D2: r0 = A0@g0, q = A0@g1 (fp8 DoubleRow), reductions ----
        # et-outer with all 8 g-tiles resident: each et's PSUM accumulator
        # completes early so its reductions overlap the next et's matmuls.
        with tc.tile_pool(name="gb", bufs=1) as gbp, \
             tc.tile_pool(name="fin", bufs=1) as fin, \
             tc.tile_pool(name="pr", bufs=1, space="PSUM") as prp:
            # q = A0@g1 first (its exchange lands first), then r0 = A0@g0
            for a_dst, gT, sq_col, dve_col, want_sumq in (
                    (ag2b_dst, g1T, 8, 4, True),
                    (ag2a_dst, g0T, 0, 0, False)):
                gpk = [gbp.tile([P, NE, E], FP8, tag=f"gpk{k}", bufs=2,
                                name=f"gpk{k}") for k in range(NCORES)]
                for k in range(NCORES):
                    nc.sync.dma_start(
                        out=gpk[k][:],
                        in_=a_dst[k * P:(k + 1) * P, :].rearrange(
                            "p (s e) -> p s e", s=NE))
                for et in range(NE):
                    rp_ = prp.tile([P, NL], F32, tag=f"r{et % 2}",
                                   bufs=2, name=f"r{et}")
                    es = slice(et * P, (et + 1) * P)
                    for k in range(NCORES):
                        for cp in range(2):
                            mp = k * 4 + 2 * cp
                            nc.tensor.matmul(
                                rp_[:], gpk[k][:, 2 * cp:2 * cp + 2, es],
                                A0T[:, mp:mp + 2, :],
                                start=(k == 0 and cp == 0),
                                stop=(k == NCORES - 1 and cp == 1),
                                perf_mode=DR)
                    scr = fin.tile([P, NL], F32, tag="scr", bufs=2, name="scr")
                    nc.scalar.activation(
                        out=scr[:], in_=rp_[:], func=AF.Square,
                        accum_out=out_act[:, sq_col + et:sq_col + et + 1])
                    scr2 = fin.tile([P, NL], F32, tag="scr2", bufs=2,
                                    name="scr2")
                    nc.vector.scalar_tensor_tensor(
                        out=scr2[:], in0=rp_[:], scalar=1.0,
                        in1=gT[:, et, :],
                        op0=ALU.mult, op1=ALU.mult,
                        accum_out=out_dve[:, dve_col + et:dve_col + et + 1])
                    if want_sumq:
                        scr3 = fin.tile([P, NL], F32, tag="scr3", bufs=2,
                                        name="scr3")
                        nc.scalar.activation(
                            out=scr3[:], in_=rp_[:], func=AF.Copy,
                            accum_out=out_act[:, 4 + et:4 + et + 1])

            nc.sync.dma_start(out=T["out_a"][:], in_=out_act[:])
            nc.sync.dma_start(out=T["out_d"][:], in_=out_dve[:])


_PROGRAM = None


def _get_program():
    global _PROGRAM
    if _PROGRAM is None:
        _PROGRAM = build_program()
    return _PROGRAM


def _prep_w(w, gates=(0, 1, 2, 3)):
    """(4H, E_in) torch-layout weight -> bf16 lhsT tiles [p, kt, len(gates)*H]."""
    import ml_dtypes
    w = np.asarray(w, np.float32)
    h4 = w.shape[0]
    h = h4 // 4
    wt = np.concatenate([w[g * h:(g + 1) * h] for g in gates], 0).T
    e_in, cols = wt.shape
    return np.ascontiguousarray(
        wt.reshape(e_in // P, P, cols).transpose(1, 0, 2)
        .astype(ml_dtypes.bfloat16))


def _prep_b(b1, b2, gates=(0, 1, 2, 3)):
    s = np.asarray(b1, np.float32) + np.asarray(b2, np.float32)
    h = s.shape[0] // 4
    s = np.concatenate([s[g * h:(g + 1) * h] for g in gates], 0)
    return np.ascontiguousarray(s.reshape(-1, P).T)


def run_device(inputs, trace=False):
    """Shard inputs, run the 8-core SPMD program, return bass results."""
    nc = _get_program()
    emb = np.ascontiguousarray(np.asarray(inputs["embedding"], np.float32))
    iq = np.asarray(inputs["input"]).astype(np.int32).reshape(N, 1)
    ie = np.asarray(inputs["set_inputs"]).astype(np.int32)
    shared = {
        "emb": emb,
        "wgf": _prep_w(inputs["wih_gf"]), "wgr": _prep_w(inputs["wih_gr"]),
        "ugf": _prep_w(inputs["whh_gf"]), "ugr": _prep_w(inputs["whh_gr"]),
        "wf": _prep_w(inputs["wih_f"], gates=(0, 2, 3)),
        "bgf": _prep_b(inputs["bih_gf"], inputs["bhh_gf"]),
        "bgr": _prep_b(inputs["bih_gr"], inputs["bhh_gr"]),
        "bf": _prep_b(inputs["bih_f"], inputs["bhh_f"], gates=(0, 2, 3)),
    }
    in_maps = []
    for k in range(NCORES):
        sl = slice(k * NL, (k + 1) * NL)
        m = dict(shared)
        m["idx_x"] = np.ascontiguousarray(iq[sl])
        m["idx_e0"] = np.ascontiguousarray(ie[0, sl].reshape(NL, 1))
        m["idx_e1"] = np.ascontiguousarray(ie[1, sl].reshape(NL, 1))
        in_maps.append(m)
    res = bass_utils.run_bass_kernel_spmd(
        nc, in_maps, core_ids=list(range(NCORES)), trace=trace)
    return res


def kernel(**inputs):
    res = run_device(inputs)
    return host_tail(res, inputs)


def host_tail(res, inputs):
    # device partial layout:
    #   out_a cols: sr0(0:4) sqq(4:8)... see below; out_d cols: dot0, dot1q
    # column c of kind at base: value for e = et*128 + p
    sr0 = np.zeros(E, np.float64)
    sqq = np.zeros(E, np.float64)
    sumq = np.zeros(E, np.float64)
    sg0 = np.zeros(E, np.float64)
    sg1 = np.zeros(E, np.float64)
    t1 = np.zeros(E, np.float64)
    dot0 = np.zeros(E, np.float64)
    dot1q = np.zeros(E, np.float64)
    for r in res.results:
        a = np.asarray(r["out_a"], np.float64)   # (P, 24)
        d = np.asarray(r["out_d"], np.float64)   # (P, 8)
        for et in range(NE):
            sl = slice(et * P, (et + 1) * P)
            sr0[sl] += a[:, 0 + et]
            sumq[sl] += a[:, 4 + et]
            sqq[sl] += a[:, 8 + et]
            sg0[sl] += a[:, 12 + et]
            sg1[sl] += a[:, 16 + et]
            t1[sl] += a[:, 20 + et]
            dot0[sl] += d[:, et]
            dot1q[sl] += d[:, 4 + et]
    s1 = t1
    dot1 = s1 * t1 - dot1q
    sr1 = N * s1 ** 2 - 2.0 * s1 * sumq + sqq
    dot = np.stack([dot0, dot1])
    sr = np.stack([sr0, sr1])
    sg = np.stack([sg0, sg1])
    nr = np.maximum(np.sqrt(sr), EPS)
    ng = np.maximum(np.sqrt(sg), EPS)
    cos = dot / (nr * ng)
    kern = cos / np.exp(cos).sum()
    w_out = np.asarray(inputs["w_out"], np.float64)
    b_out = np.asarray(inputs["b_out"], np.float64)
    k2 = kern @ w_out.T + b_out
    s = k2.sum(axis=1)
    labels = np.asarray(inputs["set_labels"], np.float64)
    o = s[0] * labels[0] + s[1] * labels[1]
    o = np.exp(o - o.max())
    o /= o.sum()
    return o.astype(np.float32)


# revision 36
# speedup vs baseline: 1.0099x; 1.0099x over previous
"""MatchingNet model kernel for 8 Trainium2 NeuronCores (v2, fp8 exchange).

Computation (reference semantics, N=4096, E=512, G=256, V=50000, R=1000):
  x  = embedding[input]          (N, E)
  ex = embedding[set_inputs]     (2, N, E)
  g_out = bidirectional 2-step LSTM over ex   (2, N, E)
  fh = lstm_f(x) + x             (N, E)          [single step, zero state]
  scores[b] = g_out[b] @ fh.T    (2, N, N)
  a = softmax(scores, axis=0)    -- softmax over b (size 2), pointwise in (n,m)
  r[b] = a[b] @ g_out[b]         (2, N, E)
  dot/nr/ng reductions over n -> cos (2, E) -> tiny tail -> softmax (R,)

Sharding: data-parallel over N. Core k owns rows [512k, 512k+512).
a[0] = sigmoid(D), a[1] = 1 - a[0] with D = (g0 - g1) @ fh.T.
r0 = A0 @ g0.  r1 = s1 - q with q = A0 @ g1 and s1[e] = sum_m g1[m,e];
the s1-dependent parts of dot1/sr1 are reconstructed on the host from
per-core partials (t1 = local colsum of g1, sum_q, sq_q, dot1q), so the
device never materializes a[1].

Exchange (all fp8 e4m3, partition-major blocks of 2KB rows):
  AG1: fh.T   (256KB/core -> 2MB)   fired right after the f-LSTM
  AG2a: g0    (256KB/core -> 2MB)   fired right after g0's transposes
  AG2b: g1    (256KB/core -> 2MB)
D1/D2 matmuls run in fp8 DoubleRow mode (2 k-subtiles per call).
"""

import os
import sys

import numpy as np

for _p in ("/opt/trn_rl_repo", os.path.expanduser("~/.axon_site/_ro/trn_rl_repo")):
    if os.path.isdir(_p) and _p not in sys.path:
        sys.path.insert(0, _p)

import concourse.bacc as bacc
import concourse.bass as bass
import concourse.mybir as mybir
import concourse.tile as tile
from concourse import bass_utils
from concourse.masks import make_identity

N, E, G, V, R = 4096, 512, 256, 50000, 1000
NCORES = 8
NL = N // NCORES  # 512 rows per core
P = 128
NE = E // P   # 4 e-chunks
NH = G // P   # 2 hidden chunks for the g-LSTM
NMB = N // P  # 32 global m-blocks
EPS = 1e-8

F32 = mybir.dt.float32
BF16 = mybir.dt.bfloat16
FP8 = mybir.dt.float8e4
I32 = mybir.dt.int32
AF = mybir.ActivationFunctionType
ALU = mybir.AluOpType
DR = mybir.MatmulPerfMode.DoubleRow


def _lstm_cell(nc, pools, H, xT, W_sb, U_sb, hprevT, cprevT, bias_sb, h_out,
               c_out, gates=(0, 1, 2, 3), packed=(0, 1, 2, 3)):
    """Emit one LSTM cell, transposed layout (feature on partition, n free).

    W_sb: (P, NE, len(packed)*H) packed in `packed` gate order; U_sb likewise
    or None.  bias_sb: (P, len(packed)*H//P).  h_out/c_out: (P, H//P, NL).
    Zero-state cells pass gates without 1 (forget) and cprevT=None.
    """
    pg, gp, tp = pools["pg"], pools["gates"], pools["tmp"]
    hc = H // P
    pos = {g: i for i, g in enumerate(packed)}
    gb = {}
    for g in gates:
        gb[g] = gp.tile([P, hc, NL], F32, tag=f"gate{g}", bufs=1, name=f"gb{g}")
        for s in range(hc):
            jc = pos[g] * hc + s
            ps = pg.tile([P, NL], F32, tag="pg", bufs=4, name="ps_gate")
            js = slice(jc * P, (jc + 1) * P)
            for kt in range(NE):
                nc.tensor.matmul(
                    ps[:], W_sb[:, kt, js], xT[:, kt, :],
                    start=(kt == 0), stop=(U_sb is None and kt == NE - 1))
            if U_sb is not None:
                hcu = hprevT.shape[1]
                for kt in range(hcu):
                    nc.tensor.matmul(
                        ps[:], U_sb[:, kt, js], hprevT[:, kt, :],
                        start=False, stop=(kt == hcu - 1))
            func = AF.Tanh if g == 2 else AF.Sigmoid
            nc.scalar.activation(
                out=gb[g][:, s, :], in_=ps[:], func=func,
                bias=bias_sb[:, jc:jc + 1], scale=1.0)
    for s in range(hc):
        i_, g_, o_ = gb[0][:, s, :], gb[2][:, s, :], gb[3][:, s, :]
        if c_out is None:
            c_s = tp.tile([P, NL], F32, tag="ctmp", bufs=2, name="ctmp")
        else:
            c_s = c_out[:, s, :]
        if cprevT is None:
            nc.vector.tensor_mul(c_s, i_, g_)
        else:
            f_ = gb[1][:, s, :]
            ig = tp.tile([P, NL], F32, tag="ig", bufs=2, name="ig")
            nc.vector.tensor_mul(ig[:], i_, g_)
            nc.vector.tensor_mul(c_s, f_, cprevT[:, s, :])
            nc.vector.tensor_add(c_s, c_s, ig[:])
        tc_ = tp.tile([P, NL], F32, tag="tanhc", bufs=2, name="tanhc")
        nc.scalar.activation(out=tc_[:], in_=c_s, func=AF.Tanh)
        nc.vector.tensor_mul(h_out[:, s, :], o_, tc_[:])


def _gather_T(nc, pools, emb, idx_dram, ident, dstT):
    """Gather NL embedding rows and transpose into dstT (P, NE, NL)."""
    ip, rp, pt, cp = pools["idx"], pools["raw"], pools["pt"], pools["tmp"]
    for t in range(NL // P):
        idx_t = ip.tile([P, 1], I32, tag="idx", bufs=4, name="idx_t")
        nc.sync.dma_start(out=idx_t[:], in_=idx_dram[t * P:(t + 1) * P, :])
        raw = rp.tile([P, E], F32, tag="raw", bufs=4, name="raw")
        nc.gpsimd.indirect_dma_start(
            out=raw[:], out_offset=None, in_=emb[:],
            in_offset=bass.IndirectOffsetOnAxis(ap=idx_t[:, :1], axis=0))
        for et in range(NE):
            ptile = pt.tile([P, P], F32, tag="pt", bufs=2, name="ptile")
            nc.tensor.transpose(
                out=ptile[:], in_=raw[:, et * P:(et + 1) * P], identity=ident[:])
            nc.vector.tensor_copy(
                out=dstT[:, et, t * P:(t + 1) * P], in_=ptile[:])


def build_program():
    nc = bacc.Bacc("TRN2", target_bir_lowering=False, debug=False,
                   enable_asserts=False, num_devices=NCORES)
    dram = lambda name, shape, dt=F32, kind="ExternalInput": \
        nc.dram_tensor(name, shape, dt, kind=kind).ap()

    emb = dram("emb", [V, E])
    idx_x = dram("idx_x", [NL, 1], I32)
    idx_e0 = dram("idx_e0", [NL, 1], I32)
    idx_e1 = dram("idx_e1", [NL, 1], I32)
    # weights pre-laid-out on host as lhsT tiles [p, kt, j]
    wgf = dram("wgf", [P, NE, 4 * G], BF16)
    wgr = dram("wgr", [P, NE, 4 * G], BF16)
    ugf = dram("ugf", [P, NH, 4 * G], BF16)
    ugr = dram("ugr", [P, NH, 4 * G], BF16)
    wf = dram("wf", [P, NE, 3 * E], BF16)   # i, g, o gates only (zero state)
    bgf = dram("bgf", [P, 8])
    bgr = dram("bgr", [P, 8])
    bf = dram("bf", [P, 12])
    out_a = dram("out_a", [P, 24], kind="ExternalOutput")
    out_d = dram("out_d", [P, 8], kind="ExternalOutput")

    with tile.TileContext(nc) as tc:
        _emit(tc, locals())
    nc.compile()
    return nc


def _emit(tc, T):
    nc = tc.nc
    rg = [list(range(NCORES))]
    from contextlib import ExitStack
    ctx = ExitStack()
    with ctx:
        glob = ctx.enter_context(tc.tile_pool(name="glob", bufs=1))
        dramp = ctx.enter_context(tc.tile_pool(name="dramp", bufs=1, space="DRAM"))

        ident = glob.tile([P, P], F32)
        make_identity(nc, ident)
        identb = glob.tile([P, P], BF16)
        nc.vector.tensor_copy(out=identb[:], in_=ident[:])

        # tiny warmup collective: primes the CC pipeline so the first real
        # AllGather runs at full bandwidth (121 vs 68 GB/s measured)
        wu_src = dramp.tile([P, 4], F32)
        wu_dst = dramp.tile([NCORES * P, 4], F32, addr_space="Shared")
        wu_sb = glob.tile([P, 4], F32)
        nc.vector.memset(wu_sb[:], 0.0)
        nc.sync.dma_start(out=wu_src[:], in_=wu_sb[:])
        nc.gpsimd.collective_compute(
            "AllGather", ALU.bypass, replica_groups=rg,
            ins=[wu_src[:].opt()], outs=[wu_dst[:].opt()])

        # fp8 exchange buffers, partition-major 2KB rows
        ag1_src = dramp.tile([P, NE * NL], FP8)
        ag1_dst = dramp.tile([NCORES * P, NE * NL], FP8, addr_space="Shared")
        ag2a_src = dramp.tile([P, 4 * E], FP8)
        ag2a_dst = dramp.tile([NCORES * P, 4 * E], FP8, addr_space="Shared")
        ag2b_src = dramp.tile([P, 4 * E], FP8)
        ag2b_dst = dramp.tile([NCORES * P, 4 * E], FP8, addr_space="Shared")

        # long-lived activations
        g0T = glob.tile([P, NE, NL], BF16)
        g1T = glob.tile([P, NE, NL], BF16)
        dg8 = glob.tile([P, NE, NL], FP8)
        A0T = glob.tile([P, NMB, NL], FP8)
        out_act = glob.tile([P, 24], F32)
        out_dve = glob.tile([P, 8], F32)

        with tc.tile_pool(name="wpool", bufs=1) as wp, \
             tc.tile_pool(name="acts", bufs=1) as ap_, \
             tc.tile_pool(name="gates", bufs=1) as gp, \
             tc.tile_pool(name="tmp", bufs=1) as tp, \
             tc.tile_pool(name="idx", bufs=1) as ip, \
             tc.tile_pool(name="raw", bufs=1) as rp, \
             tc.tile_pool(name="fhk", bufs=1) as fkp, \
             tc.tile_pool(name="tps", bufs=1) as tsp:
            lstm_psum = tc.tile_pool(name="pg", bufs=1, space="PSUM")
            pgp = lstm_psum.__enter__()
            ptp_cm = tc.tile_pool(name="pt", bufs=1, space="PSUM")
            ptp = ptp_cm.__enter__()
            pools = {"pg": pgp, "gates": gp, "tmp": tp, "idx": ip,
                     "raw": rp, "pt": ptp}

            # ---- gathers: e0 first (g-LSTM start), then x (f-LSTM has
            # slack until the barrier ends), then e1 ----
            xT = ap_.tile([P, NE, NL], BF16)
            e0T = ap_.tile([P, NE, NL], BF16)
            e1T = ap_.tile([P, NE, NL], BF16)
            _gather_T(nc, pools, T["emb"], T["idx_e0"], ident, e0T)
            _gather_T(nc, pools, T["emb"], T["idx_x"], ident, xT)
            _gather_T(nc, pools, T["emb"], T["idx_e1"], ident, e1T)

            # weight DMA order follows first use: biases, g-LSTM stage-1
            # quarters, f-LSTM, recurrent weights, forget-gate quarters
            w_sb = {}
            for nm in ("bgf", "bgr"):
                w_sb[nm] = wp.tile([P, 8], F32, name=nm + "_sb")
                nc.sync.dma_start(out=w_sb[nm][:], in_=T[nm][:])
            bf_sb = wp.tile([P, 12], F32)
            nc.sync.dma_start(out=bf_sb[:], in_=T["bf"][:])
            for nm, kt in (("wgf", NE), ("wgr", NE), ("ugf", NH), ("ugr", NH)):
                w_sb[nm] = wp.tile([P, kt, 4 * G], BF16, name=nm + "_sb")
            wf_sb = wp.tile([P, NE, 3 * E], BF16)
            for nm in ("wgf", "wgr"):
                for q in (0, 2, 3):
                    qs = slice(q * G, (q + 1) * G)
                    nc.sync.dma_start(out=w_sb[nm][:, :, qs],
                                      in_=T[nm][:, :, qs])
            for q in range(3):   # chunked so the i-gate matmuls start early
                qs = slice(q * E, (q + 1) * E)
                nc.sync.dma_start(out=wf_sb[:, :, qs], in_=T["wf"][:, :, qs])
            for nm in ("ugf", "ugr"):
                nc.sync.dma_start(out=w_sb[nm][:], in_=T[nm][:])
            for nm in ("wgf", "wgr"):   # forget gate: only stage 2 needs it
                qs = slice(1 * G, 2 * G)
                nc.sync.dma_start(out=w_sb[nm][:, :, qs], in_=T[nm][:, :, qs])

            fhT = ap_.tile([P, NE, NL], BF16)
            _lstm_cell(nc, pools, E, xT, wf_sb, None, None, None, bf_sb, fhT,
                       None, gates=(0, 2, 3), packed=(0, 2, 3))
            fh8 = ap_.tile([P, NE, NL], FP8, name="fh8")
            for et in range(NE):
                nc.vector.tensor_add(fhT[:, et, :], fhT[:, et, :], xT[:, et, :])
                nc.vector.tensor_copy(out=fh8[:, et, :], in_=fhT[:, et, :])
            nc.sync.dma_start(
                out=ag1_src[:], in_=fh8[:].rearrange("p et n -> p (et n)"))
            nc.gpsimd.collective_compute(
                "AllGather", ALU.bypass, replica_groups=rg,
                ins=[ag1_src[:].opt()], outs=[ag1_dst[:].opt()])

            # ---- g-LSTM with transposes emitted right after each cell ----
            # g1 = [hf1, hr1] completes before g0 = [hf0, hr0] (rev0 is the
            # last cell), so g1's exchange fires first and the q-phase of D2
            # runs first.  Emitting each transpose batch immediately after
            # its producing cell keeps them ahead of the AG1-gated D1 work
            # in every engine stream.
            s0_t = tsp.tile([P, NE, E], FP8, tag="s0", bufs=1, name="s0")
            s1_t = tsp.tile([P, NE, E], FP8, tag="s1", bufs=1, name="s1")

            def emit_tr(gT, e0_, s_t):
                for nt in range(NL // P):
                    ptile = ptp.tile([P, 2, P], BF16, tag="ptg", bufs=2,
                                     name="ptg")
                    for j in range(2):
                        nc.tensor.transpose(
                            out=ptile[:, j, :],
                            in_=gT[:, e0_ + j, nt * P:(nt + 1) * P],
                            identity=identb[:])
                    nc.vector.tensor_copy(
                        out=s_t[:, nt, e0_ * P:(e0_ + 2) * P], in_=ptile[:])

            cfT = ap_.tile([P, NH, NL], F32, name="cfT")
            crT = ap_.tile([P, NH, NL], F32, name="crT")
            c2T = ap_.tile([P, NH, NL], F32, name="c2T")
            c3T = ap_.tile([P, NH, NL], F32, name="c3T")
            hf0 = g0T[:, 0:NH, :]
            hf1 = g1T[:, 0:NH, :]
            hr1 = g1T[:, NH:NE, :]
            hr0 = g0T[:, NH:NE, :]
            _lstm_cell(nc, pools, G, e0T, w_sb["wgf"], None, None, None,
                       w_sb["bgf"], hf0, cfT, gates=(0, 2, 3))
            emit_tr(g0T, 0, s0_t)
            _lstm_cell(nc, pools, G, e1T, w_sb["wgr"], None, None, None,
                       w_sb["bgr"], hr1, crT, gates=(0, 2, 3))
            emit_tr(g1T, NH, s1_t)
            # forget-gate weight quarters land last; stage 2 needs them
            _lstm_cell(nc, pools, G, e1T, w_sb["wgf"], w_sb["ugf"], hf0, cfT,
                       w_sb["bgf"], hf1, c2T)
            emit_tr(g1T, 0, s1_t)
            with tc.high_priority():
                nc.sync.dma_start(
                    out=ag2b_src[:],
                    in_=s1_t[:].rearrange("p s e -> p (s e)"))
                nc.gpsimd.collective_compute(
                    "AllGather", ALU.bypass, replica_groups=rg,
                    ins=[ag2b_src[:].opt()], outs=[ag2b_dst[:].opt()])
            _lstm_cell(nc, pools, G, e0T, w_sb["wgr"], w_sb["ugr"], hr1, crT,
                       w_sb["bgr"], hr0, c3T)
            emit_tr(g0T, NH, s0_t)
            with tc.high_priority():
                nc.sync.dma_start(
                    out=ag2a_src[:],
                    in_=s0_t[:].rearrange("p s e -> p (s e)"))
                nc.gpsimd.collective_compute(
                    "AllGather", ALU.bypass, replica_groups=rg,
                    ins=[ag2a_src[:].opt()], outs=[ag2a_dst[:].opt()])
            # scheduler fence: nothing downstream (AG1-gated fhk loads, D1
            # matmuls/sigmoids, gpk loads) may be reordered ahead of the
            # transpose casts / ag2 source writes / collective fires above.
            tc.no_sync_barrier()
            for et in range(NE):
                nc.vector.tensor_sub(dg8[:, et, :], g0T[:, et, :],
                                     g1T[:, et, :])
            # sg / t1 reductions now: the Act engine is idle while AG1 runs
            for b, gT in ((0, g0T), (1, g1T)):
                for et in range(NE):
                    scr = tp.tile([P, NL], F32, tag="scr", bufs=2, name="scr")
                    c0 = 12 + 4 * b + et
                    nc.scalar.activation(
                        out=scr[:], in_=gT[:, et, :],
                        func=AF.Square, accum_out=out_act[:, c0:c0 + 1])
            for et in range(NE):
                scr = tp.tile([P, NL], F32, tag="scr", bufs=2, name="scr")
                nc.scalar.activation(
                    out=scr[:], in_=g1T[:, et, :],
                    func=AF.Copy, accum_out=out_act[:, 20 + et:20 + et + 1])
            ptp_cm.__exit__(None, None, None)
            lstm_psum.__exit__(None, None, None)
            pd_cm = tc.tile_pool(name="pd", bufs=1, space="PSUM")
            pdp = pd_cm.__enter__()

            # ---- D1: A0 = sigmoid((g0-g1) @ fh_all.T), fp8 DoubleRow ----
            # Two DMAs stage the 8 gathered fh blocks (halves pipeline D1's
            # start); single waits on the SP queue keep the scheduler from
            # interleaving AG1-gated loads ahead of the ag2 source writes.
            fhk = fkp.tile([P, NCORES, NE, NL], FP8, name="fhk_all")
            for hk in range(2):
                nc.sync.dma_start(
                    out=fhk[:, hk * 4:(hk + 1) * 4, :, :],
                    in_=ag1_dst[hk * 4 * P:(hk + 1) * 4 * P, :].rearrange(
                        "(k p) (et n) -> p k et n", p=P, et=NE))
            for k in range(NCORES):
                for c in range(0, NL // P, 2):
                    mb = k * (NL // P) + c
                    pd = pdp.tile([P, 2, NL], F32, tag="pd", bufs=2, name="pd")
                    for h in range(2):
                        cs = slice((c + h) * P, (c + h + 1) * P)
                        nc.tensor.matmul(
                            pd[:, h, :], fhk[:, k, 0:2, cs], dg8[:, 0:2, :],
                            start=True, stop=False, perf_mode=DR)
                        nc.tensor.matmul(
                            pd[:, h, :], fhk[:, k, 2:4, cs], dg8[:, 2:4, :],
                            start=False, stop=True, perf_mode=DR)
                    nc.scalar.activation(
                        out=A0T[:, mb:mb + 2, :], in_=pd[:], func=AF.Sigmoid)
            pd_cm.__exit__(None, None, None)

        # ---- D2: r0 = A0@g0, q = A0@g1 (fp8 DoubleRow), reductions ----
        # et-outer with all 8 g-tiles resident: each et's PSUM accumulator
        # completes early so its reductions overlap the next et's matmuls.
        with tc.tile_pool(name="gb", bufs=1) as gbp, \
             tc.tile_pool(name="fin", bufs=1) as fin, \
             tc.tile_pool(name="pr", bufs=1, space="PSUM") as prp:
            # q = A0@g1 first (its exchange lands first), then r0 = A0@g0
            for a_dst, gT, sq_col, dve_col, want_sumq in (
                    (ag2b_dst, g1T, 8, 4, True),
                    (ag2a_dst, g0T, 0, 0, False)):
                gpk = [gbp.tile([P, NE, E], FP8, tag=f"gpk{k}", bufs=2,
                                name=f"gpk{k}") for k in range(NCORES)]
                for k in range(NCORES):
                    nc.sync.dma_start(
                        out=gpk[k][:],
                        in_=a_dst[k * P:(k + 1) * P, :].rearrange(
                            "p (s e) -> p s e", s=NE))
                for et in range(NE):
                    rp_ = prp.tile([P, NL], F32, tag=f"r{et % 2}",
                                   bufs=2, name=f"r{et}")
                    es = slice(et * P, (et + 1) * P)
                    for k in range(NCORES):
                        for cp in range(2):
                            mp = k * 4 + 2 * cp
                            nc.tensor.matmul(
                                rp_[:], gpk[k][:, 2 * cp:2 * cp + 2, es],
                                A0T[:, mp:mp + 2, :],
                                start=(k == 0 and cp == 0),
                                stop=(k == NCORES - 1 and cp == 1),
                                perf_mode=DR)
                    scr = fin.tile([P, NL], F32, tag="scr", bufs=2, name="scr")
                    nc.scalar.activation(
                        out=scr[:], in_=rp_[:], func=AF.Square,
                        accum_out=out_act[:, sq_col + et:sq_col + et + 1])
                    scr2 = fin.tile([P, NL], F32, tag="scr2", bufs=2,
                                    name="scr2")
                    nc.vector.scalar_tensor_tensor(
                        out=scr2[:], in0=rp_[:], scalar=1.0,
                        in1=gT[:, et, :],
                        op0=ALU.mult, op1=ALU.mult,
                        accum_out=out_dve[:, dve_col + et:dve_col + et + 1])
                    if want_sumq:
                        scr3 = fin.tile([P, NL], F32, tag="scr3", bufs=2,
                                        name="scr3")
                        nc.scalar.activation(
                            out=scr3[:], in_=rp_[:], func=AF.Copy,
                            accum_out=out_act[:, 4 + et:4 + et + 1])

            nc.sync.dma_start(out=T["out_a"][:], in_=out_act[:])
            nc.sync.dma_start(out=T["out_d"][:], in_=out_dve[:])


_PROGRAM = None


def _get_program():
    global _PROGRAM
    if _PROGRAM is None:
        _PROGRAM = build_program()
    return _PROGRAM


def _prep_w(w, gates=(0, 1, 2, 3)):
    """(4H, E_in) torch-layout weight -> bf16 lhsT tiles [p, kt, len(gates)*H]."""
    import ml_dtypes
    w = np.asarray(w, np.float32)
    h4 = w.shape[0]
    h = h4 // 4
    wt = np.concatenate([w[g * h:(g + 1) * h] for g in gates], 0).T
    e_in, cols = wt.shape
    return np.ascontiguousarray(
        wt.reshape(e_in // P, P, cols).transpose(1, 0, 2)
        .astype(ml_dtypes.bfloat16))


def _prep_b(b1, b2, gates=(0, 1, 2, 3)):
    s = np.asarray(b1, np.float32) + np.asarray(b2, np.float32)
    h = s.shape[0] // 4
    s = np.concatenate([s[g * h:(g + 1) * h] for g in gates], 0)
    return np.ascontiguousarray(s.reshape(-1, P).T)


def run_device(inputs, trace=False):
    """Shard inputs, run the 8-core SPMD program, return bass results."""
    nc = _get_program()
    emb = np.ascontiguousarray(np.asarray(inputs["embedding"], np.float32))
    iq = np.asarray(inputs["input"]).astype(np.int32).reshape(N, 1)
    ie = np.asarray(inputs["set_inputs"]).astype(np.int32)
    shared = {
        "emb": emb,
        "wgf": _prep_w(inputs["wih_gf"]), "wgr": _prep_w(inputs["wih_gr"]),
        "ugf": _prep_w(inputs["whh_gf"]), "ugr": _prep_w(inputs["whh_gr"]),
        "wf": _prep_w(inputs["wih_f"], gates=(0, 2, 3)),
        "bgf": _prep_b(inputs["bih_gf"], inputs["bhh_gf"]),
        "bgr": _prep_b(inputs["bih_gr"], inputs["bhh_gr"]),
        "bf": _prep_b(inputs["bih_f"], inputs["bhh_f"], gates=(0, 2, 3)),
    }
    in_maps = []
    for k in range(NCORES):
        sl = slice(k * NL, (k + 1) * NL)
        m = dict(shared)
        m["idx_x"] = np.ascontiguousarray(iq[sl])
        m["idx_e0"] = np.ascontiguousarray(ie[0, sl].reshape(NL, 1))
        m["idx_e1"] = np.ascontiguousarray(ie[1, sl].reshape(NL, 1))
        in_maps.append(m)
    res = bass_utils.run_bass_kernel_spmd(
        nc, in_maps, core_ids=list(range(NCORES)), trace=trace)
    return res


def kernel(**inputs):
    res = run_device(inputs)
    return host_tail(res, inputs)


def host_tail(res, inputs):
    # device partial layout:
    #   out_a cols: sr0(0:4) sqq(4:8)... see below; out_d cols: dot0, dot1q
    # column c of kind at base: value for e = et*128 + p
    sr0 = np.zeros(E, np.float64)
    sqq = np.zeros(E, np.float64)
    sumq = np.zeros(E, np.float64)
    sg0 = np.zeros(E, np.float64)
    sg1 = np.zeros(E, np.float64)
    t1 = np.zeros(E, np.float64)
    dot0 = np.zeros(E, np.float64)
    dot1q = np.zeros(E, np.float64)
    for r in res.results:
        a = np.asarray(r["out_a"], np.float64)   # (P, 24)
        d = np.asarray(r["out_d"], np.float64)   # (P, 8)
        for et in range(NE):
            sl = slice(et * P, (et + 1) * P)
            sr0[sl] += a[:, 0 + et]
            sumq[sl] += a[:, 4 + et]
            sqq[sl] += a[:, 8 + et]
            sg0[sl] += a[:, 12 + et]
            sg1[sl] += a[:, 16 + et]
            t1[sl] += a[:, 20 + et]
            dot0[sl] += d[:, et]
            dot1q[sl] += d[:, 4 + et]
    s1 = t1
    dot1 = s1 * t1 - dot1q
    sr1 = N * s1 ** 2 - 2.0 * s1 * sumq + sqq
    dot = np.stack([dot0, dot1])
    sr = np.stack([sr0, sr1])
    sg = np.stack([sg0, sg1])
    nr = np.maximum(np.sqrt(sr), EPS)
    ng = np.maximum(np.sqrt(sg), EPS)
    cos = dot / (nr * ng)
    kern = cos / np.exp(cos).sum()
    w_out = np.asarray(inputs["w_out"], np.float64)
    b_out = np.asarray(inputs["b_out"], np.float64)
    k2 = kern @ w_out.T + b_out
    s = k2.sum(axis=1)
    labels = np.asarray(inputs["set_labels"], np.float64)
    o = s[0] * labels[0] + s[1] * labels[1]
    o = np.exp(o - o.max())
    o /= o.sum()
    return o.astype(np.float32)


# revision 38
# speedup vs baseline: 1.0157x; 1.0058x over previous
"""MatchingNet model kernel for 8 Trainium2 NeuronCores (v2, fp8 exchange).

Computation (reference semantics, N=4096, E=512, G=256, V=50000, R=1000):
  x  = embedding[input]          (N, E)
  ex = embedding[set_inputs]     (2, N, E)
  g_out = bidirectional 2-step LSTM over ex   (2, N, E)
  fh = lstm_f(x) + x             (N, E)          [single step, zero state]
  scores[b] = g_out[b] @ fh.T    (2, N, N)
  a = softmax(scores, axis=0)    -- softmax over b (size 2), pointwise in (n,m)
  r[b] = a[b] @ g_out[b]         (2, N, E)
  dot/nr/ng reductions over n -> cos (2, E) -> tiny tail -> softmax (R,)

Sharding: data-parallel over N. Core k owns rows [512k, 512k+512).
a[0] = sigmoid(D), a[1] = 1 - a[0] with D = (g0 - g1) @ fh.T.
r0 = A0 @ g0.  r1 = s1 - q with q = A0 @ g1 and s1[e] = sum_m g1[m,e];
the s1-dependent parts of dot1/sr1 are reconstructed on the host from
per-core partials (t1 = local colsum of g1, sum_q, sq_q, dot1q), so the
device never materializes a[1].

Exchange (all fp8 e4m3, partition-major blocks of 2KB rows):
  AG1: fh.T   (256KB/core -> 2MB)   fired right after the f-LSTM
  AG2a: g0    (256KB/core -> 2MB)   fired right after g0's transposes
  AG2b: g1    (256KB/core -> 2MB)
D1/D2 matmuls run in fp8 DoubleRow mode (2 k-subtiles per call).
"""

import os
import sys

import numpy as np

for _p in ("/opt/trn_rl_repo", os.path.expanduser("~/.axon_site/_ro/trn_rl_repo")):
    if os.path.isdir(_p) and _p not in sys.path:
        sys.path.insert(0, _p)

import concourse.bacc as bacc
import concourse.bass as bass
import concourse.mybir as mybir
import concourse.tile as tile
from concourse import bass_utils
from concourse.masks import make_identity

N, E, G, V, R = 4096, 512, 256, 50000, 1000
NCORES = 8
NL = N // NCORES  # 512 rows per core
P = 128
NE = E // P   # 4 e-chunks
NH = G // P   # 2 hidden chunks for the g-LSTM
NMB = N // P  # 32 global m-blocks
EPS = 1e-8

F32 = mybir.dt.float32
BF16 = mybir.dt.bfloat16
FP8 = mybir.dt.float8e4
I32 = mybir.dt.int32
AF = mybir.ActivationFunctionType
ALU = mybir.AluOpType
DR = mybir.MatmulPerfMode.DoubleRow


def _lstm_cell(nc, pools, H, xT, W_sb, U_sb, hprevT, cprevT, bias_sb, h_out,
               c_out, gates=(0, 1, 2, 3), packed=(0, 1, 2, 3)):
    """Emit one LSTM cell, transposed layout (feature on partition, n free).

    W_sb: (P, NE, len(packed)*H) packed in `packed` gate order; U_sb likewise
    or None.  bias_sb: (P, len(packed)*H//P).  h_out/c_out: (P, H//P, NL).
    Zero-state cells pass gates without 1 (forget) and cprevT=None.
    """
    pg, gp, tp = pools["pg"], pools["gates"], pools["tmp"]
    hc = H // P
    pos = {g: i for i, g in enumerate(packed)}
    gb = {}
    for g in gates:
        gb[g] = gp.tile([P, hc, NL], F32, tag=f"gate{g}", bufs=1, name=f"gb{g}")
        for s in range(hc):
            jc = pos[g] * hc + s
            ps = pg.tile([P, NL], F32, tag="pg", bufs=4, name="ps_gate")
            js = slice(jc * P, (jc + 1) * P)
            for kt in range(NE):
                nc.tensor.matmul(
                    ps[:], W_sb[:, kt, js], xT[:, kt, :],
                    start=(kt == 0), stop=(U_sb is None and kt == NE - 1))
            if U_sb is not None:
                hcu = hprevT.shape[1]
                for kt in range(hcu):
                    nc.tensor.matmul(
                        ps[:], U_sb[:, kt, js], hprevT[:, kt, :],
                        start=False, stop=(kt == hcu - 1))
            func = AF.Tanh if g == 2 else AF.Sigmoid
            nc.scalar.activation(
                out=gb[g][:, s, :], in_=ps[:], func=func,
                bias=bias_sb[:, jc:jc + 1], scale=1.0)
    for s in range(hc):
        i_, g_, o_ = gb[0][:, s, :], gb[2][:, s, :], gb[3][:, s, :]
        if c_out is None:
            c_s = tp.tile([P, NL], F32, tag="ctmp", bufs=2, name="ctmp")
        else:
            c_s = c_out[:, s, :]
        if cprevT is None:
            nc.vector.tensor_mul(c_s, i_, g_)
        else:
            f_ = gb[1][:, s, :]
            ig = tp.tile([P, NL], F32, tag="ig", bufs=2, name="ig")
            nc.vector.tensor_mul(ig[:], i_, g_)
            nc.vector.tensor_mul(c_s, f_, cprevT[:, s, :])
            nc.vector.tensor_add(c_s, c_s, ig[:])
        tc_ = tp.tile([P, NL], F32, tag="tanhc", bufs=2, name="tanhc")
        nc.scalar.activation(out=tc_[:], in_=c_s, func=AF.Tanh)
        nc.vector.tensor_mul(h_out[:, s, :], o_, tc_[:])


def _gather_T(nc, pools, emb, idx_dram, ident, dstT):
    """Gather NL embedding rows and transpose into dstT (P, NE, NL)."""
    ip, rp, pt, cp = pools["idx"], pools["raw"], pools["pt"], pools["tmp"]
    for t in range(NL // P):
        idx_t = ip.tile([P, 1], I32, tag="idx", bufs=4, name="idx_t")
        nc.sync.dma_start(out=idx_t[:], in_=idx_dram[t * P:(t + 1) * P, :])
        raw = rp.tile([P, E], F32, tag="raw", bufs=4, name="raw")
        nc.gpsimd.indirect_dma_start(
            out=raw[:], out_offset=None, in_=emb[:],
            in_offset=bass.IndirectOffsetOnAxis(ap=idx_t[:, :1], axis=0))
        for et in range(NE):
            ptile = pt.tile([P, P], F32, tag="pt", bufs=2, name="ptile")
            nc.tensor.transpose(
                out=ptile[:], in_=raw[:, et * P:(et + 1) * P], identity=ident[:])
            nc.vector.tensor_copy(
                out=dstT[:, et, t * P:(t + 1) * P], in_=ptile[:])


def build_program():
    nc = bacc.Bacc("TRN2", target_bir_lowering=False, debug=False,
                   enable_asserts=False, num_devices=NCORES)
    dram = lambda name, shape, dt=F32, kind="ExternalInput": \
        nc.dram_tensor(name, shape, dt, kind=kind).ap()

    emb = dram("emb", [V, E])
    idx_x = dram("idx_x", [NL, 1], I32)
    idx_e0 = dram("idx_e0", [NL, 1], I32)
    idx_e1 = dram("idx_e1", [NL, 1], I32)
    # weights pre-laid-out on host as lhsT tiles [p, kt, j]
    wgf = dram("wgf", [P, NE, 4 * G], BF16)
    wgr = dram("wgr", [P, NE, 4 * G], BF16)
    ugf = dram("ugf", [P, NH, 4 * G], BF16)
    ugr = dram("ugr", [P, NH, 4 * G], BF16)
    wf = dram("wf", [P, NE, 3 * E], BF16)   # i, g, o gates only (zero state)
    bgf = dram("bgf", [P, 8])
    bgr = dram("bgr", [P, 8])
    bf = dram("bf", [P, 12])
    out_a = dram("out_a", [P, 24], kind="ExternalOutput")
    out_d = dram("out_d", [P, 8], kind="ExternalOutput")

    with tile.TileContext(nc) as tc:
        _emit(tc, locals())
    nc.compile()
    return nc


def _emit(tc, T):
    nc = tc.nc
    rg = [list(range(NCORES))]
    from contextlib import ExitStack
    ctx = ExitStack()
    with ctx:
        glob = ctx.enter_context(tc.tile_pool(name="glob", bufs=1))
        dramp = ctx.enter_context(tc.tile_pool(name="dramp", bufs=1, space="DRAM"))

        ident = glob.tile([P, P], F32)
        make_identity(nc, ident)
        identb = glob.tile([P, P], BF16)
        nc.vector.tensor_copy(out=identb[:], in_=ident[:])

        # tiny warmup collective: primes the CC pipeline so the first real
        # AllGather runs at full bandwidth (121 vs 68 GB/s measured)
        wu_src = dramp.tile([P, 4], F32)
        wu_dst = dramp.tile([NCORES * P, 4], F32, addr_space="Shared")
        wu_sb = glob.tile([P, 4], F32)
        nc.vector.memset(wu_sb[:], 0.0)
        nc.sync.dma_start(out=wu_src[:], in_=wu_sb[:])
        nc.gpsimd.collective_compute(
            "AllGather", ALU.bypass, replica_groups=rg,
            ins=[wu_src[:].opt()], outs=[wu_dst[:].opt()])

        # fp8 exchange buffers, partition-major 2KB rows
        ag1_src = dramp.tile([P, NE * NL], FP8)
        ag1_dst = dramp.tile([NCORES * P, NE * NL], FP8, addr_space="Shared")
        ag2a_src = dramp.tile([P, 4 * E], FP8)
        ag2a_dst = dramp.tile([NCORES * P, 4 * E], FP8, addr_space="Shared")
        ag2b_src = dramp.tile([P, 4 * E], FP8)
        ag2b_dst = dramp.tile([NCORES * P, 4 * E], FP8, addr_space="Shared")

        # long-lived activations
        g0T = glob.tile([P, NE, NL], BF16)
        g1T = glob.tile([P, NE, NL], BF16)
        dg8 = glob.tile([P, NE, NL], FP8)
        A0T = glob.tile([P, NMB, NL], FP8)
        out_act = glob.tile([P, 24], F32)
        out_dve = glob.tile([P, 8], F32)

        with tc.tile_pool(name="wpool", bufs=1) as wp, \
             tc.tile_pool(name="acts", bufs=1) as ap_, \
             tc.tile_pool(name="gates", bufs=1) as gp, \
             tc.tile_pool(name="tmp", bufs=1) as tp, \
             tc.tile_pool(name="idx", bufs=1) as ip, \
             tc.tile_pool(name="raw", bufs=1) as rp, \
             tc.tile_pool(name="fhk", bufs=1) as fkp, \
             tc.tile_pool(name="tps", bufs=1) as tsp:
            lstm_psum = tc.tile_pool(name="pg", bufs=1, space="PSUM")
            pgp = lstm_psum.__enter__()
            ptp_cm = tc.tile_pool(name="pt", bufs=1, space="PSUM")
            ptp = ptp_cm.__enter__()
            pools = {"pg": pgp, "gates": gp, "tmp": tp, "idx": ip,
                     "raw": rp, "pt": ptp}

            # ---- phase F: x gather, f-LSTM, AG1 as early as possible ----
            xT = ap_.tile([P, NE, NL], BF16)
            e0T = ap_.tile([P, NE, NL], BF16)
            e1T = ap_.tile([P, NE, NL], BF16)
            _gather_T(nc, pools, T["emb"], T["idx_x"], ident, xT)
            _gather_T(nc, pools, T["emb"], T["idx_e0"], ident, e0T)
            _gather_T(nc, pools, T["emb"], T["idx_e1"], ident, e1T)

            wf_sb = wp.tile([P, NE, 3 * E], BF16)
            for q in range(3):   # chunked so the i-gate matmuls start early
                qs = slice(q * E, (q + 1) * E)
                nc.sync.dma_start(out=wf_sb[:, :, qs], in_=T["wf"][:, :, qs])
            bf_sb = wp.tile([P, 12], F32)
            nc.sync.dma_start(out=bf_sb[:], in_=T["bf"][:])
            w_sb = {}
            for nm, kt in (("wgf", NE), ("wgr", NE), ("ugf", NH), ("ugr", NH)):
                w_sb[nm] = wp.tile([P, kt, 4 * G], BF16, name=nm + "_sb")
                for q in (0, 2, 3, 1):   # forget gate last (needed by stage 2)
                    qs = slice(q * G, (q + 1) * G)
                    nc.sync.dma_start(out=w_sb[nm][:, :, qs],
                                      in_=T[nm][:, :, qs])
            for nm in ("bgf", "bgr"):
                w_sb[nm] = wp.tile([P, 8], F32, name=nm + "_sb")
                nc.sync.dma_start(out=w_sb[nm][:], in_=T[nm][:])

            fhT = ap_.tile([P, NE, NL], BF16)
            _lstm_cell(nc, pools, E, xT, wf_sb, None, None, None, bf_sb, fhT,
                       None, gates=(0, 2, 3), packed=(0, 2, 3))
            fh8 = ap_.tile([P, NE, NL], FP8, name="fh8")
            for et in range(NE):
                nc.vector.tensor_add(fhT[:, et, :], fhT[:, et, :], xT[:, et, :])
                nc.vector.tensor_copy(out=fh8[:, et, :], in_=fhT[:, et, :])
            nc.sync.dma_start(
                out=ag1_src[:], in_=fh8[:].rearrange("p et n -> p (et n)"))
            nc.gpsimd.collective_compute(
                "AllGather", ALU.bypass, replica_groups=rg,
                ins=[ag1_src[:].opt()], outs=[ag1_dst[:].opt()])

            # ---- g-LSTM with transposes emitted right after each cell ----
            # g1 = [hf1, hr1] completes before g0 = [hf0, hr0] (rev0 is the
            # last cell), so g1's exchange fires first and the q-phase of D2
            # runs first.  Emitting each transpose batch immediately after
            # its producing cell keeps them ahead of the AG1-gated D1 work
            # in every engine stream.
            s0_t = tsp.tile([P, NE, E], FP8, tag="s0", bufs=1, name="s0")
            s1_t = tsp.tile([P, NE, E], FP8, tag="s1", bufs=1, name="s1")

            def emit_tr(gT, e0_, s_t):
                for nt in range(NL // P):
                    ptile = ptp.tile([P, 2, P], BF16, tag="ptg", bufs=2,
                                     name="ptg")
                    for j in range(2):
                        nc.tensor.transpose(
                            out=ptile[:, j, :],
                            in_=gT[:, e0_ + j, nt * P:(nt + 1) * P],
                            identity=identb[:])
                    nc.vector.tensor_copy(
                        out=s_t[:, nt, e0_ * P:(e0_ + 2) * P], in_=ptile[:])

            cfT = ap_.tile([P, NH, NL], F32, name="cfT")
            crT = ap_.tile([P, NH, NL], F32, name="crT")
            c2T = ap_.tile([P, NH, NL], F32, name="c2T")
            c3T = ap_.tile([P, NH, NL], F32, name="c3T")
            hf0 = g0T[:, 0:NH, :]
            hf1 = g1T[:, 0:NH, :]
            hr1 = g1T[:, NH:NE, :]
            hr0 = g0T[:, NH:NE, :]
            _lstm_cell(nc, pools, G, e0T, w_sb["wgf"], None, None, None,
                       w_sb["bgf"], hf0, cfT, gates=(0, 2, 3))
            emit_tr(g0T, 0, s0_t)
            _lstm_cell(nc, pools, G, e1T, w_sb["wgr"], None, None, None,
                       w_sb["bgr"], hr1, crT, gates=(0, 2, 3))
            emit_tr(g1T, NH, s1_t)
            # forget-gate weight quarters land last; stage 2 needs them
            _lstm_cell(nc, pools, G, e1T, w_sb["wgf"], w_sb["ugf"], hf0, cfT,
                       w_sb["bgf"], hf1, c2T)
            emit_tr(g1T, 0, s1_t)
            with tc.high_priority():
                nc.sync.dma_start(
                    out=ag2b_src[:],
                    in_=s1_t[:].rearrange("p s e -> p (s e)"))
                nc.gpsimd.collective_compute(
                    "AllGather", ALU.bypass, replica_groups=rg,
                    ins=[ag2b_src[:].opt()], outs=[ag2b_dst[:].opt()])
            _lstm_cell(nc, pools, G, e0T, w_sb["wgr"], w_sb["ugr"], hr1, crT,
                       w_sb["bgr"], hr0, c3T)
            emit_tr(g0T, NH, s0_t)
            with tc.high_priority():
                nc.sync.dma_start(
                    out=ag2a_src[:],
                    in_=s0_t[:].rearrange("p s e -> p (s e)"))
                nc.gpsimd.collective_compute(
                    "AllGather", ALU.bypass, replica_groups=rg,
                    ins=[ag2a_src[:].opt()], outs=[ag2a_dst[:].opt()])
            # scheduler fence: nothing downstream (AG1-gated fhk loads, D1
            # matmuls/sigmoids, gpk loads) may be reordered ahead of the
            # transpose casts / ag2 source writes / collective fires above.
            tc.no_sync_barrier()
            for et in range(NE):
                nc.vector.tensor_sub(dg8[:, et, :], g0T[:, et, :],
                                     g1T[:, et, :])
            # sg / t1 reductions now: the Act engine is idle while AG1 runs
            for b, gT in ((0, g0T), (1, g1T)):
                for et in range(NE):
                    scr = tp.tile([P, NL], F32, tag="scr", bufs=2, name="scr")
                    c0 = 12 + 4 * b + et
                    nc.scalar.activation(
                        out=scr[:], in_=gT[:, et, :],
                        func=AF.Square, accum_out=out_act[:, c0:c0 + 1])
            for et in range(NE):
                scr = tp.tile([P, NL], F32, tag="scr", bufs=2, name="scr")
                nc.scalar.activation(
                    out=scr[:], in_=g1T[:, et, :],
                    func=AF.Copy, accum_out=out_act[:, 20 + et:20 + et + 1])
            ptp_cm.__exit__(None, None, None)
            lstm_psum.__exit__(None, None, None)
            pd_cm = tc.tile_pool(name="pd", bufs=1, space="PSUM")
            pdp = pd_cm.__enter__()

            # ---- D1: A0 = sigmoid((g0-g1) @ fh_all.T), fp8 DoubleRow ----
            # Two DMAs stage the 8 gathered fh blocks (halves pipeline D1's
            # start); single waits on the SP queue keep the scheduler from
            # interleaving AG1-gated loads ahead of the ag2 source writes.
            fhk = fkp.tile([P, NCORES, NE, NL], FP8, name="fhk_all")
            for hk in range(2):
                nc.sync.dma_start(
                    out=fhk[:, hk * 4:(hk + 1) * 4, :, :],
                    in_=ag1_dst[hk * 4 * P:(hk + 1) * 4 * P, :].rearrange(
                        "(k p) (et n) -> p k et n", p=P, et=NE))
            for k in range(NCORES):
                for c in range(0, NL // P, 2):
                    mb = k * (NL // P) + c
                    pd = pdp.tile([P, 2, NL], F32, tag="pd", bufs=2, name="pd")
                    for h in range(2):
                        cs = slice((c + h) * P, (c + h + 1) * P)
                        nc.tensor.matmul(
                            pd[:, h, :], fhk[:, k, 0:2, cs], dg8[:, 0:2, :],
                            start=True, stop=False, perf_mode=DR)
                        nc.tensor.matmul(
                            pd[:, h, :], fhk[:, k, 2:4, cs], dg8[:, 2:4, :],
                            start=False, stop=True, perf_mode=DR)
                    nc.scalar.activation(
                        out=A0T[:, mb:mb + 2, :], in_=pd[:], func=AF.Sigmoid)
            pd_cm.__exit__(None, None, None)

        # ---- D2: r0 = A0@g0, q = A0@g1 (fp8 DoubleRow), reductions ----
        # et-outer with all 8 g-tiles resident: each et's PSUM accumulator
        # completes early so its reductions overlap the next et's matmuls.
        with tc.tile_pool(name="gb", bufs=1) as gbp, \
             tc.tile_pool(name="fin", bufs=1) as fin, \
             tc.tile_pool(name="pr", bufs=1, space="PSUM") as prp:
            # q = A0@g1 first (its exchange lands first), then r0 = A0@g0
            for a_dst, gT, sq_col, dve_col, want_sumq in (
                    (ag2b_dst, g1T, 8, 4, True),
                    (ag2a_dst, g0T, 0, 0, False)):
                gpk = [gbp.tile([P, NE, E], FP8, tag=f"gpk{k}", bufs=2,
                                name=f"gpk{k}") for k in range(NCORES)]
                for k in range(NCORES):
                    nc.sync.dma_start(
                        out=gpk[k][:],
                        in_=a_dst[k * P:(k + 1) * P, :].rearrange(
                            "p (s e) -> p s e", s=NE))
                for et in range(NE):
                    rp_ = prp.tile([P, NL], F32, tag=f"r{et % 2}",
                                   bufs=2, name=f"r{et}")
                    es = slice(et * P, (et + 1) * P)
                    for k in range(NCORES):
                        for cp in range(2):
                            mp = k * 4 + 2 * cp
                            nc.tensor.matmul(
                                rp_[:], gpk[k][:, 2 * cp:2 * cp + 2, es],
                                A0T[:, mp:mp + 2, :],
                                start=(k == 0 and cp == 0),
                                stop=(k == NCORES - 1 and cp == 1),
                                perf_mode=DR)
                    scr = fin.tile([P, NL], F32, tag="scr", bufs=2, name="scr")
                    nc.scalar.activation(
                        out=scr[:], in_=rp_[:], func=AF.Square,
                        accum_out=out_act[:, sq_col + et:sq_col + et + 1])
                    scr2 = fin.tile([P, NL], F32, tag="scr2", bufs=2,
                                    name="scr2")
                    nc.vector.scalar_tensor_tensor(
                        out=scr2[:], in0=rp_[:], scalar=1.0,
                        in1=gT[:, et, :],
                        op0=ALU.mult, op1=ALU.mult,
                        accum_out=out_dve[:, dve_col + et:dve_col + et + 1])
                    if want_sumq:
                        scr3 = fin.tile([P, NL], F32, tag="scr3", bufs=2,
                                        name="scr3")
                        nc.scalar.activation(
                            out=scr3[:], in_=rp_[:], func=AF.Copy,
                            accum_out=out_act[:, 4 + et:4 + et + 1])

            nc.sync.dma_start(out=T["out_a"][:], in_=out_act[:])
            nc.sync.dma_start(out=T["out_d"][:], in_=out_dve[:])


_PROGRAM = None


def _get_program():
    global _PROGRAM
    if _PROGRAM is None:
        _PROGRAM = build_program()
    return _PROGRAM


def _prep_w(w, gates=(0, 1, 2, 3)):
    """(4H, E_in) torch-layout weight -> bf16 lhsT tiles [p, kt, len(gates)*H]."""
    import ml_dtypes
    w = np.asarray(w, np.float32)
    h4 = w.shape[0]
    h = h4 // 4
    wt = np.concatenate([w[g * h:(g + 1) * h] for g in gates], 0).T
    e_in, cols = wt.shape
    return np.ascontiguousarray(
        wt.reshape(e_in // P, P, cols).transpose(1, 0, 2)
        .astype(ml_dtypes.bfloat16))


def _prep_b(b1, b2, gates=(0, 1, 2, 3)):
    s = np.asarray(b1, np.float32) + np.asarray(b2, np.float32)
    h = s.shape[0] // 4
    s = np.concatenate([s[g * h:(g + 1) * h] for g in gates], 0)
    return np.ascontiguousarray(s.reshape(-1, P).T)


def run_device(inputs, trace=False):
    """Shard inputs, run the 8-core SPMD program, return bass results."""
    nc = _get_program()
    emb = np.ascontiguousarray(np.asarray(inputs["embedding"], np.float32))
    iq = np.asarray(inputs["input"]).astype(np.int32).reshape(N, 1)
    ie = np.asarray(inputs["set_inputs"]).astype(np.int32)
    shared = {
        "emb": emb,
        "wgf": _prep_w(inputs["wih_gf"]), "wgr": _prep_w(inputs["wih_gr"]),
        "ugf": _prep_w(inputs["whh_gf"]), "ugr": _prep_w(inputs["whh_gr"]),
        "wf": _prep_w(inputs["wih_f"], gates=(0, 2, 3)),
        "bgf": _prep_b(inputs["bih_gf"], inputs["bhh_gf"]),
        "bgr": _prep_b(inputs["bih_gr"], inputs["bhh_gr"]),
        "bf": _prep_b(inputs["bih_f"], inputs["bhh_f"], gates=(0, 2, 3)),
    }
    in_maps = []
    for k in range(NCORES):
        sl = slice(k * NL, (k + 1) * NL)
        m = dict(shared)
        m["idx_x"] = np.ascontiguousarray(iq[sl])
        m["idx_e0"] = np.ascontiguousarray(ie[0, sl].reshape(NL, 1))
        m["idx_e1"] = np.ascontiguousarray(ie[1, sl].reshape(NL, 1))
        in_maps.append(m)
    res = bass_utils.run_bass_kernel_spmd(
        nc, in_maps, core_ids=list(range(NCORES)), trace=trace)
    return res


def kernel(**inputs):
    res = run_device(inputs)
    return host_tail(res, inputs)


def host_tail(res, inputs):
    # device partial layout:
    #   out_a cols: sr0(0:4) sqq(4:8)... see below; out_d cols: dot0, dot1q
    # column c of kind at base: value for e = et*128 + p
    sr0 = np.zeros(E, np.float64)
    sqq = np.zeros(E, np.float64)
    sumq = np.zeros(E, np.float64)
    sg0 = np.zeros(E, np.float64)
    sg1 = np.zeros(E, np.float64)
    t1 = np.zeros(E, np.float64)
    dot0 = np.zeros(E, np.float64)
    dot1q = np.zeros(E, np.float64)
    for r in res.results:
        a = np.asarray(r["out_a"], np.float64)   # (P, 24)
        d = np.asarray(r["out_d"], np.float64)   # (P, 8)
        for et in range(NE):
            sl = slice(et * P, (et + 1) * P)
            sr0[sl] += a[:, 0 + et]
            sumq[sl] += a[:, 4 + et]
            sqq[sl] += a[:, 8 + et]
            sg0[sl] += a[:, 12 + et]
            sg1[sl] += a[:, 16 + et]
            t1[sl] += a[:, 20 + et]
            dot0[sl] += d[:, et]
            dot1q[sl] += d[:, 4 + et]
    s1 = t1
    dot1 = s1 * t1 - dot1q
    sr1 = N * s1 ** 2 - 2.0 * s1 * sumq + sqq
    dot = np.stack([dot0, dot1])
    sr = np.stack([sr0, sr1])
    sg = np.stack([sg0, sg1])
    nr = np.maximum(np.sqrt(sr), EPS)
    ng = np.maximum(np.sqrt(sg), EPS)
    cos = dot / (nr * ng)
    kern = cos / np.exp(cos).sum()
    w_out = np.asarray(inputs["w_out"], np.float64)
    b_out = np.asarray(inputs["b_out"], np.float64)
    k2 = kern @ w_out.T + b_out
    s = k2.sum(axis=1)
    labels = np.asarray(inputs["set_labels"], np.float64)
    o = s[0] * labels[0] + s[1] * labels[1]
    o = np.exp(o - o.max())
    o /= o.sum()
    return o.astype(np.float32)


# revision 43
# speedup vs baseline: 1.0268x; 1.0109x over previous
"""MatchingNet model kernel for 8 Trainium2 NeuronCores (v2, fp8 exchange).

Computation (reference semantics, N=4096, E=512, G=256, V=50000, R=1000):
  x  = embedding[input]          (N, E)
  ex = embedding[set_inputs]     (2, N, E)
  g_out = bidirectional 2-step LSTM over ex   (2, N, E)
  fh = lstm_f(x) + x             (N, E)          [single step, zero state]
  scores[b] = g_out[b] @ fh.T    (2, N, N)
  a = softmax(scores, axis=0)    -- softmax over b (size 2), pointwise in (n,m)
  r[b] = a[b] @ g_out[b]         (2, N, E)
  dot/nr/ng reductions over n -> cos (2, E) -> tiny tail -> softmax (R,)

Sharding: data-parallel over N. Core k owns rows [512k, 512k+512).
a[0] = sigmoid(D), a[1] = 1 - a[0] with D = (g0 - g1) @ fh.T.
r0 = A0 @ g0.  r1 = s1 - q with q = A0 @ g1 and s1[e] = sum_m g1[m,e];
the s1-dependent parts of dot1/sr1 are reconstructed on the host from
per-core partials (t1 = local colsum of g1, sum_q, sq_q, dot1q), so the
device never materializes a[1].

Exchange (all fp8 e4m3, partition-major blocks of 2KB rows):
  AG1: fh.T   (256KB/core -> 2MB)   fired right after the f-LSTM
  AG2a: g0    (256KB/core -> 2MB)   fired right after g0's transposes
  AG2b: g1    (256KB/core -> 2MB)
D1/D2 matmuls run in fp8 DoubleRow mode (2 k-subtiles per call).
"""

import os
import sys

import numpy as np

for _p in ("/opt/trn_rl_repo", os.path.expanduser("~/.axon_site/_ro/trn_rl_repo")):
    if os.path.isdir(_p) and _p not in sys.path:
        sys.path.insert(0, _p)

import concourse.bacc as bacc
import concourse.bass as bass
import concourse.mybir as mybir
import concourse.tile as tile
from concourse import bass_utils
from concourse.masks import make_identity

N, E, G, V, R = 4096, 512, 256, 50000, 1000
NCORES = 8
NL = N // NCORES  # 512 rows per core
P = 128
NE = E // P   # 4 e-chunks
NH = G // P   # 2 hidden chunks for the g-LSTM
NMB = N // P  # 32 global m-blocks
EPS = 1e-8

F32 = mybir.dt.float32
BF16 = mybir.dt.bfloat16
FP8 = mybir.dt.float8e4
I32 = mybir.dt.int32
AF = mybir.ActivationFunctionType
ALU = mybir.AluOpType
DR = mybir.MatmulPerfMode.DoubleRow


def _lstm_cell(nc, pools, H, xT, W_sb, U_sb, hprevT, cprevT, bias_sb, h_out,
               c_out, gates=(0, 1, 2, 3), packed=(0, 1, 2, 3)):
    """Emit one LSTM cell, transposed layout (feature on partition, n free).

    W_sb: (P, NE, len(packed)*H) packed in `packed` gate order; U_sb likewise
    or None.  bias_sb: (P, len(packed)*H//P).  h_out/c_out: (P, H//P, NL).
    Zero-state cells pass gates without 1 (forget) and cprevT=None.
    """
    pg, gp, tp = pools["pg"], pools["gates"], pools["tmp"]
    hc = H // P
    pos = {g: i for i, g in enumerate(packed)}
    gb = {}
    for g in gates:
        gb[g] = gp.tile([P, hc, NL], F32, tag=f"gate{g}", bufs=1, name=f"gb{g}")
        for s in range(hc):
            jc = pos[g] * hc + s
            ps = pg.tile([P, NL], F32, tag="pg", bufs=4, name="ps_gate")
            js = slice(jc * P, (jc + 1) * P)
            for kt in range(NE):
                nc.tensor.matmul(
                    ps[:], W_sb[:, kt, js], xT[:, kt, :],
                    start=(kt == 0), stop=(U_sb is None and kt == NE - 1))
            if U_sb is not None:
                hcu = hprevT.shape[1]
                for kt in range(hcu):
                    nc.tensor.matmul(
                        ps[:], U_sb[:, kt, js], hprevT[:, kt, :],
                        start=False, stop=(kt == hcu - 1))
            func = AF.Tanh if g == 2 else AF.Sigmoid
            nc.scalar.activation(
                out=gb[g][:, s, :], in_=ps[:], func=func,
                bias=bias_sb[:, jc:jc + 1], scale=1.0)
    for s in range(hc):
        i_, g_, o_ = gb[0][:, s, :], gb[2][:, s, :], gb[3][:, s, :]
        if c_out is None:
            c_s = tp.tile([P, NL], F32, tag="ctmp", bufs=2, name="ctmp")
        else:
            c_s = c_out[:, s, :]
        if cprevT is None:
            nc.vector.tensor_mul(c_s, i_, g_)
        else:
            f_ = gb[1][:, s, :]
            ig = tp.tile([P, NL], F32, tag="ig", bufs=2, name="ig")
            nc.vector.tensor_mul(ig[:], i_, g_)
            nc.vector.tensor_mul(c_s, f_, cprevT[:, s, :])
            nc.vector.tensor_add(c_s, c_s, ig[:])
        tc_ = tp.tile([P, NL], F32, tag="tanhc", bufs=2, name="tanhc")
        nc.scalar.activation(out=tc_[:], in_=c_s, func=AF.Tanh)
        nc.vector.tensor_mul(h_out[:, s, :], o_, tc_[:])


def _gather_T(nc, pools, emb, idx_dram, ident, dstT):
    """Gather NL embedding rows and transpose into dstT (P, NE, NL)."""
    ip, rp, pt, cp = pools["idx"], pools["raw"], pools["pt"], pools["tmp"]
    for t in range(NL // P):
        idx_t = ip.tile([P, 1], I32, tag="idx", bufs=4, name="idx_t")
        nc.sync.dma_start(out=idx_t[:], in_=idx_dram[t * P:(t + 1) * P, :])
        raw = rp.tile([P, E], F32, tag="raw", bufs=4, name="raw")
        nc.gpsimd.indirect_dma_start(
            out=raw[:], out_offset=None, in_=emb[:],
            in_offset=bass.IndirectOffsetOnAxis(ap=idx_t[:, :1], axis=0))
        for et in range(NE):
            ptile = pt.tile([P, P], F32, tag="pt", bufs=2, name="ptile")
            nc.tensor.transpose(
                out=ptile[:], in_=raw[:, et * P:(et + 1) * P], identity=ident[:])
            nc.vector.tensor_copy(
                out=dstT[:, et, t * P:(t + 1) * P], in_=ptile[:])


def build_program():
    nc = bacc.Bacc("TRN2", target_bir_lowering=False, debug=False,
                   enable_asserts=False, num_devices=NCORES)
    dram = lambda name, shape, dt=F32, kind="ExternalInput": \
        nc.dram_tensor(name, shape, dt, kind=kind).ap()

    emb = dram("emb", [V, E])
    idx_x = dram("idx_x", [NL, 1], I32)
    idx_e0 = dram("idx_e0", [NL, 1], I32)
    idx_e1 = dram("idx_e1", [NL, 1], I32)
    # weights pre-laid-out on host as lhsT tiles [p, kt, j]
    wgf = dram("wgf", [P, NE, 4 * G], BF16)
    wgr = dram("wgr", [P, NE, 4 * G], BF16)
    ugf = dram("ugf", [P, NH, 4 * G], BF16)
    ugr = dram("ugr", [P, NH, 4 * G], BF16)
    wf = dram("wf", [P, NE, 3 * E], BF16)   # i, g, o gates only (zero state)
    bgf = dram("bgf", [P, 8])
    bgr = dram("bgr", [P, 8])
    bf = dram("bf", [P, 12])
    out_a = dram("out_a", [P, 24], kind="ExternalOutput")
    out_d = dram("out_d", [P, 8], kind="ExternalOutput")

    with tile.TileContext(nc) as tc:
        _emit(tc, locals())
    nc.compile()
    return nc


def _emit(tc, T):
    nc = tc.nc
    rg = [list(range(NCORES))]
    from contextlib import ExitStack
    ctx = ExitStack()
    with ctx:
        glob = ctx.enter_context(tc.tile_pool(name="glob", bufs=1))
        dramp = ctx.enter_context(tc.tile_pool(name="dramp", bufs=1, space="DRAM"))

        ident = glob.tile([P, P], F32)
        make_identity(nc, ident)
        identb = glob.tile([P, P], BF16)
        nc.vector.tensor_copy(out=identb[:], in_=ident[:])

        # tiny warmup collective: primes the CC pipeline so the first real
        # AllGather runs at full bandwidth (121 vs 68 GB/s measured)
        wu_src = dramp.tile([P, 4], F32)
        wu_dst = dramp.tile([NCORES * P, 4], F32, addr_space="Shared")
        wu_sb = glob.tile([P, 4], F32)
        nc.vector.memset(wu_sb[:], 0.0)
        nc.sync.dma_start(out=wu_src[:], in_=wu_sb[:])
        nc.gpsimd.collective_compute(
            "AllGather", ALU.bypass, replica_groups=rg,
            ins=[wu_src[:].opt()], outs=[wu_dst[:].opt()])

        # fp8 exchange buffers, partition-major 2KB rows
        ag1_src = dramp.tile([P, NE * NL], FP8)
        ag1_dst = dramp.tile([NCORES * P, NE * NL], FP8, addr_space="Shared")
        # g0 goes out in two half-gathers so the r0 matmuls on the first
        # half overlap the second half's transfer (r0 is the exposed tail)
        ag2a_src1 = dramp.tile([P, 2 * E], FP8)
        ag2a_dst1 = dramp.tile([NCORES * P, 2 * E], FP8, addr_space="Shared")
        ag2a_src2 = dramp.tile([P, 2 * E], FP8)
        ag2a_dst2 = dramp.tile([NCORES * P, 2 * E], FP8, addr_space="Shared")
        ag2b_src = dramp.tile([P, 4 * E], FP8)
        ag2b_dst = dramp.tile([NCORES * P, 4 * E], FP8, addr_space="Shared")

        # long-lived activations
        g0T = glob.tile([P, NE, NL], BF16)
        g1T = glob.tile([P, NE, NL], BF16)
        dg8 = glob.tile([P, NE, NL], FP8)
        A0T = glob.tile([P, NMB, NL], FP8)
        out_act = glob.tile([P, 24], F32)
        out_dve = glob.tile([P, 8], F32)

        with tc.tile_pool(name="wpool", bufs=1) as wp, \
             tc.tile_pool(name="acts", bufs=1) as ap_, \
             tc.tile_pool(name="gates", bufs=1) as gp, \
             tc.tile_pool(name="tmp", bufs=1) as tp, \
             tc.tile_pool(name="idx", bufs=1) as ip, \
             tc.tile_pool(name="raw", bufs=1) as rp, \
             tc.tile_pool(name="fhk", bufs=1) as fkp, \
             tc.tile_pool(name="tps", bufs=1) as tsp:
            lstm_psum = tc.tile_pool(name="pg", bufs=1, space="PSUM")
            pgp = lstm_psum.__enter__()
            ptp_cm = tc.tile_pool(name="pt", bufs=1, space="PSUM")
            ptp = ptp_cm.__enter__()
            pools = {"pg": pgp, "gates": gp, "tmp": tp, "idx": ip,
                     "raw": rp, "pt": ptp}

            # ---- phase F: x gather, f-LSTM, AG1 as early as possible ----
            xT = ap_.tile([P, NE, NL], BF16)
            e0T = ap_.tile([P, NE, NL], BF16)
            e1T = ap_.tile([P, NE, NL], BF16)
            _gather_T(nc, pools, T["emb"], T["idx_x"], ident, xT)
            _gather_T(nc, pools, T["emb"], T["idx_e0"], ident, e0T)
            _gather_T(nc, pools, T["emb"], T["idx_e1"], ident, e1T)

            wf_sb = wp.tile([P, NE, 3 * E], BF16)
            for q in range(3):   # chunked so the i-gate matmuls start early
                qs = slice(q * E, (q + 1) * E)
                nc.sync.dma_start(out=wf_sb[:, :, qs], in_=T["wf"][:, :, qs])
            bf_sb = wp.tile([P, 12], F32)
            nc.sync.dma_start(out=bf_sb[:], in_=T["bf"][:])
            w_sb = {}
            for nm, kt in (("wgf", NE), ("wgr", NE), ("ugf", NH), ("ugr", NH)):
                w_sb[nm] = wp.tile([P, kt, 4 * G], BF16, name=nm + "_sb")
                for q in (0, 2, 3, 1):   # forget gate last (needed by stage 2)
                    qs = slice(q * G, (q + 1) * G)
                    nc.sync.dma_start(out=w_sb[nm][:, :, qs],
                                      in_=T[nm][:, :, qs])
            for nm in ("bgf", "bgr"):
                w_sb[nm] = wp.tile([P, 8], F32, name=nm + "_sb")
                nc.sync.dma_start(out=w_sb[nm][:], in_=T[nm][:])

            fhT = ap_.tile([P, NE, NL], BF16)
            _lstm_cell(nc, pools, E, xT, wf_sb, None, None, None, bf_sb, fhT,
                       None, gates=(0, 2, 3), packed=(0, 2, 3))
            fh8 = ap_.tile([P, NE, NL], FP8, name="fh8")
            for et in range(NE):
                nc.vector.tensor_add(fhT[:, et, :], fhT[:, et, :], xT[:, et, :])
                nc.vector.tensor_copy(out=fh8[:, et, :], in_=fhT[:, et, :])
            nc.sync.dma_start(
                out=ag1_src[:], in_=fh8[:].rearrange("p et n -> p (et n)"))
            nc.gpsimd.collective_compute(
                "AllGather", ALU.bypass, replica_groups=rg,
                ins=[ag1_src[:].opt()], outs=[ag1_dst[:].opt()])

            # ---- g-LSTM with transposes emitted right after each cell ----
            # g1 = [hf1, hr1] completes before g0 = [hf0, hr0] (rev0 is the
            # last cell), so g1's exchange fires first and the q-phase of D2
            # runs first.  Emitting each transpose batch immediately after
            # its producing cell keeps them ahead of the AG1-gated D1 work
            # in every engine stream.
            s0_t = tsp.tile([P, NE, E], FP8, tag="s0", bufs=1, name="s0")
            s1_t = tsp.tile([P, NE, E], FP8, tag="s1", bufs=1, name="s1")

            def emit_tr(gT, e0_, s_t):
                for nt in range(NL // P):
                    ptile = ptp.tile([P, 2, P], BF16, tag="ptg", bufs=2,
                                     name="ptg")
                    for j in range(2):
                        nc.tensor.transpose(
                            out=ptile[:, j, :],
                            in_=gT[:, e0_ + j, nt * P:(nt + 1) * P],
                            identity=identb[:])
                    nc.vector.tensor_copy(
                        out=s_t[:, nt, e0_ * P:(e0_ + 2) * P], in_=ptile[:])

            cfT = ap_.tile([P, NH, NL], F32, name="cfT")
            crT = ap_.tile([P, NH, NL], F32, name="crT")
            c2T = ap_.tile([P, NH, NL], F32, name="c2T")
            c3T = ap_.tile([P, NH, NL], F32, name="c3T")
            hf0 = g0T[:, 0:NH, :]
            hf1 = g1T[:, 0:NH, :]
            hr1 = g1T[:, NH:NE, :]
            hr0 = g0T[:, NH:NE, :]
            _lstm_cell(nc, pools, G, e0T, w_sb["wgf"], None, None, None,
                       w_sb["bgf"], hf0, cfT, gates=(0, 2, 3))
            emit_tr(g0T, 0, s0_t)
            _lstm_cell(nc, pools, G, e1T, w_sb["wgr"], None, None, None,
                       w_sb["bgr"], hr1, crT, gates=(0, 2, 3))
            emit_tr(g1T, NH, s1_t)
            # forget-gate weight quarters land last; stage 2 needs them
            _lstm_cell(nc, pools, G, e1T, w_sb["wgf"], w_sb["ugf"], hf0, cfT,
                       w_sb["bgf"], hf1, c2T)
            emit_tr(g1T, 0, s1_t)
            with tc.high_priority():
                nc.sync.dma_start(
                    out=ag2b_src[:],
                    in_=s1_t[:].rearrange("p s e -> p (s e)"))
                nc.gpsimd.collective_compute(
                    "AllGather", ALU.bypass, replica_groups=rg,
                    ins=[ag2b_src[:].opt()], outs=[ag2b_dst[:].opt()])
            _lstm_cell(nc, pools, G, e0T, w_sb["wgr"], w_sb["ugr"], hr1, crT,
                       w_sb["bgr"], hr0, c3T)
            emit_tr(g0T, NH, s0_t)
            with tc.high_priority():
                nc.sync.dma_start(
                    out=ag2a_src1[:],
                    in_=s0_t[:, 0:2, :].rearrange("p s e -> p (s e)"))
                nc.gpsimd.collective_compute(
                    "AllGather", ALU.bypass, replica_groups=rg,
                    ins=[ag2a_src1[:].opt()], outs=[ag2a_dst1[:].opt()])
                nc.sync.dma_start(
                    out=ag2a_src2[:],
                    in_=s0_t[:, 2:4, :].rearrange("p s e -> p (s e)"))
                nc.gpsimd.collective_compute(
                    "AllGather", ALU.bypass, replica_groups=rg,
                    ins=[ag2a_src2[:].opt()], outs=[ag2a_dst2[:].opt()])
            # scheduler fence: nothing downstream (AG1-gated fhk loads, D1
            # matmuls/sigmoids, gpk loads) may be reordered ahead of the
            # transpose casts / ag2 source writes / collective fires above.
            tc.no_sync_barrier()
            for et in range(NE):
                nc.vector.tensor_sub(dg8[:, et, :], g0T[:, et, :],
                                     g1T[:, et, :])
            # sg / t1 reductions now: the Act engine is idle while AG1 runs
            for b, gT in ((0, g0T), (1, g1T)):
                for et in range(NE):
                    scr = tp.tile([P, NL], F32, tag="scr", bufs=2, name="scr")
                    c0 = 12 + 4 * b + et
                    nc.scalar.activation(
                        out=scr[:], in_=gT[:, et, :],
                        func=AF.Square, accum_out=out_act[:, c0:c0 + 1])
            for et in range(NE):
                scr = tp.tile([P, NL], F32, tag="scr", bufs=2, name="scr")
                nc.scalar.activation(
                    out=scr[:], in_=g1T[:, et, :],
                    func=AF.Copy, accum_out=out_act[:, 20 + et:20 + et + 1])
            ptp_cm.__exit__(None, None, None)
            lstm_psum.__exit__(None, None, None)
            pd_cm = tc.tile_pool(name="pd", bufs=1, space="PSUM")
            pdp = pd_cm.__enter__()

            # ---- D1: A0 = sigmoid((g0-g1) @ fh_all.T), fp8 DoubleRow ----
            # Two DMAs stage the 8 gathered fh blocks (halves pipeline D1's
            # start); single waits on the SP queue keep the scheduler from
            # interleaving AG1-gated loads ahead of the ag2 source writes.
            fhk = fkp.tile([P, NCORES, NE, NL], FP8, name="fhk_all")
            for hk in range(4):
                nc.sync.dma_start(
                    out=fhk[:, hk * 2:(hk + 1) * 2, :, :],
                    in_=ag1_dst[hk * 2 * P:(hk + 1) * 2 * P, :].rearrange(
                        "(k p) (et n) -> p k et n", p=P, et=NE))
            for k in range(NCORES):
                for c in range(0, NL // P, 2):
                    mb = k * (NL // P) + c
                    pd = pdp.tile([P, 2, NL], F32, tag="pd", bufs=2, name="pd")
                    for h in range(2):
                        cs = slice((c + h) * P, (c + h + 1) * P)
                        nc.tensor.matmul(
                            pd[:, h, :], fhk[:, k, 0:2, cs], dg8[:, 0:2, :],
                            start=True, stop=False, perf_mode=DR)
                        nc.tensor.matmul(
                            pd[:, h, :], fhk[:, k, 2:4, cs], dg8[:, 2:4, :],
                            start=False, stop=True, perf_mode=DR)
                    nc.scalar.activation(
                        out=A0T[:, mb:mb + 2, :], in_=pd[:], func=AF.Sigmoid)
            pd_cm.__exit__(None, None, None)

        # ---- D2: r0 = A0@g0, q = A0@g1 (fp8 DoubleRow), reductions ----
        # et-outer with all 8 g-tiles resident: each et's PSUM accumulator
        # completes early so its reductions overlap the next et's matmuls.
        with tc.tile_pool(name="gb", bufs=1) as gbp, \
             tc.tile_pool(name="fin", bufs=1) as fin, \
             tc.tile_pool(name="pr", bufs=1, space="PSUM") as prp:
            def reduce_et(rp_, gT, et, sq_col, dve_col, want_sumq):
                scr = fin.tile([P, NL], F32, tag="scr", bufs=2, name="scr")
                nc.scalar.activation(
                    out=scr[:], in_=rp_[:], func=AF.Square,
                    accum_out=out_act[:, sq_col + et:sq_col + et + 1])
                scr2 = fin.tile([P, NL], F32, tag="scr2", bufs=2, name="scr2")
                nc.vector.scalar_tensor_tensor(
                    out=scr2[:], in0=rp_[:], scalar=1.0, in1=gT[:, et, :],
                    op0=ALU.mult, op1=ALU.mult,
                    accum_out=out_dve[:, dve_col + et:dve_col + et + 1])
                if want_sumq:
                    scr3 = fin.tile([P, NL], F32, tag="scr3", bufs=2,
                                    name="scr3")
                    nc.scalar.activation(
                        out=scr3[:], in_=rp_[:], func=AF.Copy,
                        accum_out=out_act[:, 4 + et:4 + et + 1])

            # q = A0@g1 first (its exchange lands first)
            gpk = [gbp.tile([P, NE, E], FP8, tag=f"gpk{k}", bufs=1,
                            name=f"gpk{k}") for k in range(NCORES)]
            for k in range(NCORES):
                nc.sync.dma_start(
                    out=gpk[k][:],
                    in_=ag2b_dst[k * P:(k + 1) * P, :].rearrange(
                        "p (s e) -> p s e", s=NE))
            for et in range(NE):
                rp_ = prp.tile([P, NL], F32, tag=f"r{et % 2}",
                               bufs=2, name=f"r{et}")
                es = slice(et * P, (et + 1) * P)
                for k in range(NCORES):
                    for cp in range(2):
                        mp = k * 4 + 2 * cp
                        nc.tensor.matmul(
                            rp_[:], gpk[k][:, 2 * cp:2 * cp + 2, es],
                            A0T[:, mp:mp + 2, :],
                            start=(k == 0 and cp == 0),
                            stop=(k == NCORES - 1 and cp == 1),
                            perf_mode=DR)
                reduce_et(rp_, g1T, et, 8, 4, True)

            # r0 = A0@g0: first half (m-subtiles 0,1 of each block) arrives
            # one half-gather earlier — its matmuls overlap the second half
            gplo = [gbp.tile([P, 2, E], FP8, tag=f"glo{k}", bufs=1,
                             name=f"glo{k}") for k in range(NCORES)]
            gphi = [gbp.tile([P, 2, E], FP8, tag=f"ghi{k}", bufs=1,
                             name=f"ghi{k}") for k in range(NCORES)]
            for k in range(NCORES):
                nc.sync.dma_start(
                    out=gplo[k][:],
                    in_=ag2a_dst1[k * P:(k + 1) * P, :].rearrange(
                        "p (s e) -> p s e", s=2))
            for k in range(NCORES):
                nc.sync.dma_start(
                    out=gphi[k][:],
                    in_=ag2a_dst2[k * P:(k + 1) * P, :].rearrange(
                        "p (s e) -> p s e", s=2))
            rps = [prp.tile([P, NL], F32, tag=f"r{et % 2}", bufs=2,
                            name=f"r0_{et}") for et in range(NE)]
            for et in range(NE):
                es = slice(et * P, (et + 1) * P)
                for k in range(NCORES):
                    nc.tensor.matmul(
                        rps[et][:], gplo[k][:, :, es],
                        A0T[:, k * 4:k * 4 + 2, :],
                        start=(k == 0), stop=False, perf_mode=DR)
            for et in range(NE):
                es = slice(et * P, (et + 1) * P)
                for k in range(NCORES):
                    nc.tensor.matmul(
                        rps[et][:], gphi[k][:, :, es],
                        A0T[:, k * 4 + 2:k * 4 + 4, :],
                        start=False, stop=(k == NCORES - 1), perf_mode=DR)
                reduce_et(rps[et], g0T, et, 0, 0, False)

            nc.sync.dma_start(out=T["out_a"][:], in_=out_act[:])
            nc.sync.dma_start(out=T["out_d"][:], in_=out_dve[:])


_PROGRAM = None


def _get_program():
    global _PROGRAM
    if _PROGRAM is None:
        _PROGRAM = build_program()
    return _PROGRAM


def _prep_w(w, gates=(0, 1, 2, 3)):
    """(4H, E_in) torch-layout weight -> bf16 lhsT tiles [p, kt, len(gates)*H]."""
    import ml_dtypes
    w = np.asarray(w, np.float32)
    h4 = w.shape[0]
    h = h4 // 4
    wt = np.concatenate([w[g * h:(g + 1) * h] for g in gates], 0).T
    e_in, cols = wt.shape
    return np.ascontiguousarray(
        wt.reshape(e_in // P, P, cols).transpose(1, 0, 2)
        .astype(ml_dtypes.bfloat16))


def _prep_b(b1, b2, gates=(0, 1, 2, 3)):
    s = np.asarray(b1, np.float32) + np.asarray(b2, np.float32)
    h = s.shape[0] // 4
    s = np.concatenate([s[g * h:(g + 1) * h] for g in gates], 0)
    return np.ascontiguousarray(s.reshape(-1, P).T)


def run_device(inputs, trace=False):
    """Shard inputs, run the 8-core SPMD program, return bass results."""
    nc = _get_program()
    emb = np.ascontiguousarray(np.asarray(inputs["embedding"], np.float32))
    iq = np.asarray(inputs["input"]).astype(np.int32).reshape(N, 1)
    ie = np.asarray(inputs["set_inputs"]).astype(np.int32)
    shared = {
        "emb": emb,
        "wgf": _prep_w(inputs["wih_gf"]), "wgr": _prep_w(inputs["wih_gr"]),
        "ugf": _prep_w(inputs["whh_gf"]), "ugr": _prep_w(inputs["whh_gr"]),
        "wf": _prep_w(inputs["wih_f"], gates=(0, 2, 3)),
        "bgf": _prep_b(inputs["bih_gf"], inputs["bhh_gf"]),
        "bgr": _prep_b(inputs["bih_gr"], inputs["bhh_gr"]),
        "bf": _prep_b(inputs["bih_f"], inputs["bhh_f"], gates=(0, 2, 3)),
    }
    in_maps = []
    for k in range(NCORES):
        sl = slice(k * NL, (k + 1) * NL)
        m = dict(shared)
        m["idx_x"] = np.ascontiguousarray(iq[sl])
        m["idx_e0"] = np.ascontiguousarray(ie[0, sl].reshape(NL, 1))
        m["idx_e1"] = np.ascontiguousarray(ie[1, sl].reshape(NL, 1))
        in_maps.append(m)
    res = bass_utils.run_bass_kernel_spmd(
        nc, in_maps, core_ids=list(range(NCORES)), trace=trace)
    return res


def kernel(**inputs):
    res = run_device(inputs)
    return host_tail(res, inputs)


def host_tail(res, inputs):
    # device partial layout:
    #   out_a cols: sr0(0:4) sqq(4:8)... see below; out_d cols: dot0, dot1q
    # column c of kind at base: value for e = et*128 + p
    sr0 = np.zeros(E, np.float64)
    sqq = np.zeros(E, np.float64)
    sumq = np.zeros(E, np.float64)
    sg0 = np.zeros(E, np.float64)
    sg1 = np.zeros(E, np.float64)
    t1 = np.zeros(E, np.float64)
    dot0 = np.zeros(E, np.float64)
    dot1q = np.zeros(E, np.float64)
    for r in res.results:
        a = np.asarray(r["out_a"], np.float64)   # (P, 24)
        d = np.asarray(r["out_d"], np.float64)   # (P, 8)
        for et in range(NE):
            sl = slice(et * P, (et + 1) * P)
            sr0[sl] += a[:, 0 + et]
            sumq[sl] += a[:, 4 + et]
            sqq[sl] += a[:, 8 + et]
            sg0[sl] += a[:, 12 + et]
            sg1[sl] += a[:, 16 + et]
            t1[sl] += a[:, 20 + et]
            dot0[sl] += d[:, et]
            dot1q[sl] += d[:, 4 + et]
    s1 = t1
    dot1 = s1 * t1 - dot1q
    sr1 = N * s1 ** 2 - 2.0 * s1 * sumq + sqq
    dot = np.stack([dot0, dot1])
    sr = np.stack([sr0, sr1])
    sg = np.stack([sg0, sg1])
    nr = np.maximum(np.sqrt(sr), EPS)
    ng = np.maximum(np.sqrt(sg), EPS)
    cos = dot / (nr * ng)
    kern = cos / np.exp(cos).sum()
    w_out = np.asarray(inputs["w_out"], np.float64)
    b_out = np.asarray(inputs["b_out"], np.float64)
    k2 = kern @ w_out.T + b_out
    s = k2.sum(axis=1)
    labels = np.asarray(inputs["set_labels"], np.float64)
    o = s[0] * labels[0] + s[1] * labels[1]
    o = np.exp(o - o.max())
    o /= o.sum()
    return o.astype(np.float32)


# revision 47
# speedup vs baseline: 1.0912x; 1.0627x over previous
"""MatchingNet model kernel for 8 Trainium2 NeuronCores (v2, fp8 exchange).

Computation (reference semantics, N=4096, E=512, G=256, V=50000, R=1000):
  x  = embedding[input]          (N, E)
  ex = embedding[set_inputs]     (2, N, E)
  g_out = bidirectional 2-step LSTM over ex   (2, N, E)
  fh = lstm_f(x) + x             (N, E)          [single step, zero state]
  scores[b] = g_out[b] @ fh.T    (2, N, N)
  a = softmax(scores, axis=0)    -- softmax over b (size 2), pointwise in (n,m)
  r[b] = a[b] @ g_out[b]         (2, N, E)
  dot/nr/ng reductions over n -> cos (2, E) -> tiny tail -> softmax (R,)

Sharding: data-parallel over N. Core k owns rows [512k, 512k+512).
a[0] = sigmoid(D), a[1] = 1 - a[0] with D = (g0 - g1) @ fh.T.
r0 = A0 @ g0.  r1 = s1 - q with q = A0 @ g1 and s1[e] = sum_m g1[m,e];
the s1-dependent parts of dot1/sr1 are reconstructed on the host from
per-core partials (t1 = local colsum of g1, sum_q, sq_q, dot1q), so the
device never materializes a[1].

Exchange (all fp8 e4m3, partition-major blocks of 2KB rows):
  AG1: fh.T   (256KB/core -> 2MB)   fired right after the f-LSTM
  AG2a: g0    (256KB/core -> 2MB)   fired right after g0's transposes
  AG2b: g1    (256KB/core -> 2MB)
D1/D2 matmuls run in fp8 DoubleRow mode (2 k-subtiles per call).
"""

import os
import sys

import numpy as np

for _p in ("/opt/trn_rl_repo", os.path.expanduser("~/.axon_site/_ro/trn_rl_repo")):
    if os.path.isdir(_p) and _p not in sys.path:
        sys.path.insert(0, _p)

import concourse.bacc as bacc
import concourse.bass as bass
import concourse.mybir as mybir
import concourse.tile as tile
from concourse import bass_utils
from concourse.masks import make_identity

N, E, G, V, R = 4096, 512, 256, 50000, 1000
NCORES = 8
NL = N // NCORES  # 512 rows per core
P = 128
NE = E // P   # 4 e-chunks
NH = G // P   # 2 hidden chunks for the g-LSTM
NMB = N // P  # 32 global m-blocks
EPS = 1e-8

F32 = mybir.dt.float32
BF16 = mybir.dt.bfloat16
FP8 = mybir.dt.float8e4
I32 = mybir.dt.int32
AF = mybir.ActivationFunctionType
ALU = mybir.AluOpType
DR = mybir.MatmulPerfMode.DoubleRow


def _lstm_cell(nc, pools, H, xT, W_sb, U_sb, hprevT, cprevT, bias_sb, h_out,
               c_out, gates=(0, 1, 2, 3), packed=(0, 1, 2, 3)):
    """Emit one LSTM cell, transposed layout (feature on partition, n free).

    W_sb: (P, NE, len(packed)*H) packed in `packed` gate order; U_sb likewise
    or None.  bias_sb: (P, len(packed)*H//P).  h_out/c_out: (P, H//P, NL).
    Zero-state cells pass gates without 1 (forget) and cprevT=None.
    """
    pg, gp, tp = pools["pg"], pools["gates"], pools["tmp"]
    hc = H // P
    pos = {g: i for i, g in enumerate(packed)}
    gb = {}
    for g in gates:
        gb[g] = gp.tile([P, hc, NL], F32, tag=f"gate{g}", bufs=1, name=f"gb{g}")
        for s in range(hc):
            jc = pos[g] * hc + s
            ps = pg.tile([P, NL], F32, tag="pg", bufs=4, name="ps_gate")
            js = slice(jc * P, (jc + 1) * P)
            for kt in range(NE):
                nc.tensor.matmul(
                    ps[:], W_sb[:, kt, js], xT[:, kt, :],
                    start=(kt == 0), stop=(U_sb is None and kt == NE - 1))
            if U_sb is not None:
                hcu = hprevT.shape[1]
                for kt in range(hcu):
                    nc.tensor.matmul(
                        ps[:], U_sb[:, kt, js], hprevT[:, kt, :],
                        start=False, stop=(kt == hcu - 1))
            func = AF.Tanh if g == 2 else AF.Sigmoid
            nc.scalar.activation(
                out=gb[g][:, s, :], in_=ps[:], func=func,
                bias=bias_sb[:, jc:jc + 1], scale=1.0)
    for s in range(hc):
        i_, g_, o_ = gb[0][:, s, :], gb[2][:, s, :], gb[3][:, s, :]
        if c_out is None:
            c_s = tp.tile([P, NL], F32, tag="ctmp", bufs=2, name="ctmp")
        else:
            c_s = c_out[:, s, :]
        if cprevT is None:
            nc.vector.tensor_mul(c_s, i_, g_)
        else:
            f_ = gb[1][:, s, :]
            ig = tp.tile([P, NL], F32, tag="ig", bufs=2, name="ig")
            nc.vector.tensor_mul(ig[:], i_, g_)
            nc.vector.tensor_mul(c_s, f_, cprevT[:, s, :])
            nc.vector.tensor_add(c_s, c_s, ig[:])
        tc_ = tp.tile([P, NL], F32, tag="tanhc", bufs=2, name="tanhc")
        nc.scalar.activation(out=tc_[:], in_=c_s, func=AF.Tanh)
        nc.vector.tensor_mul(h_out[:, s, :], o_, tc_[:])


def _gather_T(nc, pools, emb, idx_dram, ident, dstT):
    """Gather NL embedding rows and transpose into dstT (P, NE, NL)."""
    ip, rp, pt, cp = pools["idx"], pools["raw"], pools["pt"], pools["tmp"]
    for t in range(NL // P):
        idx_t = ip.tile([P, 1], I32, tag="idx", bufs=4, name="idx_t")
        nc.sync.dma_start(out=idx_t[:], in_=idx_dram[t * P:(t + 1) * P, :])
        raw = rp.tile([P, E], F32, tag="raw", bufs=4, name="raw")
        nc.gpsimd.indirect_dma_start(
            out=raw[:], out_offset=None, in_=emb[:],
            in_offset=bass.IndirectOffsetOnAxis(ap=idx_t[:, :1], axis=0))
        for et in range(NE):
            ptile = pt.tile([P, P], F32, tag="pt", bufs=2, name="ptile")
            nc.tensor.transpose(
                out=ptile[:], in_=raw[:, et * P:(et + 1) * P], identity=ident[:])
            nc.vector.tensor_copy(
                out=dstT[:, et, t * P:(t + 1) * P], in_=ptile[:])


def build_program():
    nc = bacc.Bacc("TRN2", target_bir_lowering=False, debug=False,
                   enable_asserts=False, num_devices=NCORES)
    dram = lambda name, shape, dt=F32, kind="ExternalInput": \
        nc.dram_tensor(name, shape, dt, kind=kind).ap()

    emb = dram("emb", [V, E])
    idx_x = dram("idx_x", [NL, 1], I32)
    idx_e0 = dram("idx_e0", [NL, 1], I32)
    idx_e1 = dram("idx_e1", [NL, 1], I32)
    # weights pre-laid-out on host as lhsT tiles [p, kt, j]
    wgf = dram("wgf", [P, NE, 4 * G], BF16)
    wgr = dram("wgr", [P, NE, 4 * G], BF16)
    ugf = dram("ugf", [P, NH, 4 * G], BF16)
    ugr = dram("ugr", [P, NH, 4 * G], BF16)
    wf = dram("wf", [P, NE, 3 * E], BF16)   # i, g, o gates only (zero state)
    bgf = dram("bgf", [P, 8])
    bgr = dram("bgr", [P, 8])
    bf = dram("bf", [P, 12])
    out_a = dram("out_a", [P, 24], kind="ExternalOutput")
    out_d = dram("out_d", [P, 8], kind="ExternalOutput")

    with tile.TileContext(nc) as tc:
        _emit(tc, locals())
    nc.compile()
    return nc


def _emit(tc, T):
    nc = tc.nc
    rg = [list(range(NCORES))]
    from contextlib import ExitStack
    ctx = ExitStack()
    with ctx:
        glob = ctx.enter_context(tc.tile_pool(name="glob", bufs=1))
        dramp = ctx.enter_context(tc.tile_pool(name="dramp", bufs=1, space="DRAM"))

        ident = glob.tile([P, P], F32)
        make_identity(nc, ident)
        identb = glob.tile([P, P], BF16)
        nc.vector.tensor_copy(out=identb[:], in_=ident[:])

        # tiny warmup collective: primes the CC pipeline so the first real
        # AllGather runs at full bandwidth (121 vs 68 GB/s measured)
        wu_src = dramp.tile([P, 4], F32)
        wu_dst = dramp.tile([NCORES * P, 4], F32, addr_space="Shared")
        wu_sb = glob.tile([P, 4], F32)
        nc.vector.memset(wu_sb[:], 0.0)
        nc.sync.dma_start(out=wu_src[:], in_=wu_sb[:])
        nc.gpsimd.collective_compute(
            "AllGather", ALU.bypass, replica_groups=rg,
            ins=[wu_src[:].opt()], outs=[wu_dst[:].opt()])

        # fp8 exchange buffers, partition-major 2KB rows
        ag1_src = dramp.tile([P, NE * NL], FP8)
        ag1_dst = dramp.tile([NCORES * P, NE * NL], FP8, addr_space="Shared")
        ag2a_src = dramp.tile([P, 4 * E], FP8)
        ag2a_dst = dramp.tile([NCORES * P, 4 * E], FP8, addr_space="Shared")
        ag2b_src = dramp.tile([P, 4 * E], FP8)
        ag2b_dst = dramp.tile([NCORES * P, 4 * E], FP8, addr_space="Shared")

        # long-lived activations
        g0T = glob.tile([P, NE, NL], BF16)
        g1T = glob.tile([P, NE, NL], BF16)
        dg8 = glob.tile([P, NE, NL], FP8)
        A0T = glob.tile([P, NMB, NL], FP8)
        out_act = glob.tile([P, 24], F32)
        out_dve = glob.tile([P, 8], F32)

        with tc.tile_pool(name="wpool", bufs=1) as wp, \
             tc.tile_pool(name="acts", bufs=1) as ap_, \
             tc.tile_pool(name="gates", bufs=1) as gp, \
             tc.tile_pool(name="tmp", bufs=1) as tp, \
             tc.tile_pool(name="idx", bufs=1) as ip, \
             tc.tile_pool(name="raw", bufs=1) as rp, \
             tc.tile_pool(name="fhk", bufs=1) as fkp, \
             tc.tile_pool(name="tps", bufs=1) as tsp:
            lstm_psum = tc.tile_pool(name="pg", bufs=1, space="PSUM")
            pgp = lstm_psum.__enter__()
            ptp_cm = tc.tile_pool(name="pt", bufs=1, space="PSUM")
            ptp = ptp_cm.__enter__()
            pools = {"pg": pgp, "gates": gp, "tmp": tp, "idx": ip,
                     "raw": rp, "pt": ptp}

            # ---- phase F: x gather, f-LSTM, AG1 as early as possible ----
            xT = ap_.tile([P, NE, NL], BF16)
            e0T = ap_.tile([P, NE, NL], BF16)
            e1T = ap_.tile([P, NE, NL], BF16)
            _gather_T(nc, pools, T["emb"], T["idx_x"], ident, xT)
            _gather_T(nc, pools, T["emb"], T["idx_e0"], ident, e0T)
            _gather_T(nc, pools, T["emb"], T["idx_e1"], ident, e1T)

            wf_sb = wp.tile([P, NE, 3 * E], BF16)
            for q in range(3):   # chunked so the i-gate matmuls start early
                qs = slice(q * E, (q + 1) * E)
                nc.sync.dma_start(out=wf_sb[:, :, qs], in_=T["wf"][:, :, qs])
            bf_sb = wp.tile([P, 12], F32)
            nc.sync.dma_start(out=bf_sb[:], in_=T["bf"][:])
            w_sb = {}
            for nm, kt in (("wgf", NE), ("wgr", NE), ("ugf", NH), ("ugr", NH)):
                w_sb[nm] = wp.tile([P, kt, 4 * G], BF16, name=nm + "_sb")
                for q in (0, 2, 3, 1):   # forget gate last (needed by stage 2)
                    qs = slice(q * G, (q + 1) * G)
                    nc.sync.dma_start(out=w_sb[nm][:, :, qs],
                                      in_=T[nm][:, :, qs])
            for nm in ("bgf", "bgr"):
                w_sb[nm] = wp.tile([P, 8], F32, name=nm + "_sb")
                nc.sync.dma_start(out=w_sb[nm][:], in_=T[nm][:])

            fhT = ap_.tile([P, NE, NL], BF16)
            _lstm_cell(nc, pools, E, xT, wf_sb, None, None, None, bf_sb, fhT,
                       None, gates=(0, 2, 3), packed=(0, 2, 3))
            fh8 = ap_.tile([P, NE, NL], FP8, name="fh8")
            for et in range(NE):
                nc.vector.tensor_add(fhT[:, et, :], fhT[:, et, :], xT[:, et, :])
                nc.vector.tensor_copy(out=fh8[:, et, :], in_=fhT[:, et, :])
            nc.sync.dma_start(
                out=ag1_src[:], in_=fh8[:].rearrange("p et n -> p (et n)"))
            nc.gpsimd.collective_compute(
                "AllGather", ALU.bypass, replica_groups=rg,
                ins=[ag1_src[:].opt()], outs=[ag1_dst[:].opt()])

            # ---- g-LSTM with transposes emitted right after each cell ----
            # g1 = [hf1, hr1] completes before g0 = [hf0, hr0] (rev0 is the
            # last cell), so g1's exchange fires first and the q-phase of D2
            # runs first.  Emitting each transpose batch immediately after
            # its producing cell keeps them ahead of the AG1-gated D1 work
            # in every engine stream.
            s0_t = tsp.tile([P, NE, E], FP8, tag="s0", bufs=1, name="s0")
            s1_t = tsp.tile([P, NE, E], FP8, tag="s1", bufs=1, name="s1")

            def emit_tr(gT, e0_, s_t):
                for nt in range(NL // P):
                    ptile = ptp.tile([P, 2, P], BF16, tag="ptg", bufs=2,
                                     name="ptg")
                    for j in range(2):
                        nc.tensor.transpose(
                            out=ptile[:, j, :],
                            in_=gT[:, e0_ + j, nt * P:(nt + 1) * P],
                            identity=identb[:])
                    nc.vector.tensor_copy(
                        out=s_t[:, nt, e0_ * P:(e0_ + 2) * P], in_=ptile[:])

            cfT = ap_.tile([P, NH, NL], F32, name="cfT")
            crT = ap_.tile([P, NH, NL], F32, name="crT")
            c2T = ap_.tile([P, NH, NL], F32, name="c2T")
            c3T = ap_.tile([P, NH, NL], F32, name="c3T")
            hf0 = g0T[:, 0:NH, :]
            hf1 = g1T[:, 0:NH, :]
            hr1 = g1T[:, NH:NE, :]
            hr0 = g0T[:, NH:NE, :]
            _lstm_cell(nc, pools, G, e0T, w_sb["wgf"], None, None, None,
                       w_sb["bgf"], hf0, cfT, gates=(0, 2, 3))
            emit_tr(g0T, 0, s0_t)
            _lstm_cell(nc, pools, G, e1T, w_sb["wgr"], None, None, None,
                       w_sb["bgr"], hr1, crT, gates=(0, 2, 3))
            emit_tr(g1T, NH, s1_t)
            # forget-gate weight quarters land last; stage 2 needs them
            _lstm_cell(nc, pools, G, e1T, w_sb["wgf"], w_sb["ugf"], hf0, cfT,
                       w_sb["bgf"], hf1, c2T)
            emit_tr(g1T, 0, s1_t)
            with tc.high_priority():
                nc.sync.dma_start(
                    out=ag2b_src[:],
                    in_=s1_t[:].rearrange("p s e -> p (s e)"))
                nc.gpsimd.collective_compute(
                    "AllGather", ALU.bypass, replica_groups=rg,
                    ins=[ag2b_src[:].opt()], outs=[ag2b_dst[:].opt()])
            _lstm_cell(nc, pools, G, e0T, w_sb["wgr"], w_sb["ugr"], hr1, crT,
                       w_sb["bgr"], hr0, c3T)
            emit_tr(g0T, NH, s0_t)
            with tc.high_priority():
                nc.sync.dma_start(
                    out=ag2a_src[:],
                    in_=s0_t[:].rearrange("p s e -> p (s e)"))
                nc.gpsimd.collective_compute(
                    "AllGather", ALU.bypass, replica_groups=rg,
                    ins=[ag2a_src[:].opt()], outs=[ag2a_dst[:].opt()])
            # scheduler fence: nothing downstream (AG1-gated fhk loads, D1
            # matmuls/sigmoids, gpk loads) may be reordered ahead of the
            # transpose casts / ag2 source writes / collective fires above.
            tc.no_sync_barrier()
            for et in range(NE):
                nc.vector.tensor_sub(dg8[:, et, :], g0T[:, et, :],
                                     g1T[:, et, :])
            # sg / t1 reductions now: the Act engine is idle while AG1 runs
            for b, gT in ((0, g0T), (1, g1T)):
                for et in range(NE):
                    scr = tp.tile([P, NL], F32, tag="scr", bufs=2, name="scr")
                    c0 = 12 + 4 * b + et
                    nc.scalar.activation(
                        out=scr[:], in_=gT[:, et, :],
                        func=AF.Square, accum_out=out_act[:, c0:c0 + 1])
            for et in range(NE):
                scr = tp.tile([P, NL], F32, tag="scr", bufs=2, name="scr")
                nc.scalar.activation(
                    out=scr[:], in_=g1T[:, et, :],
                    func=AF.Copy, accum_out=out_act[:, 20 + et:20 + et + 1])
            ptp_cm.__exit__(None, None, None)
            lstm_psum.__exit__(None, None, None)
            pd_cm = tc.tile_pool(name="pd", bufs=1, space="PSUM")
            pdp = pd_cm.__enter__()

            # ---- D1: A0 = sigmoid((g0-g1) @ fh_all.T), fp8 DoubleRow ----
            # Two DMAs stage the 8 gathered fh blocks (halves pipeline D1's
            # start); single waits on the SP queue keep the scheduler from
            # interleaving AG1-gated loads ahead of the ag2 source writes.
            fhk = fkp.tile([P, NCORES, NE, NL], FP8, name="fhk_all")
            for hk in range(2):
                nc.sync.dma_start(
                    out=fhk[:, hk * 4:(hk + 1) * 4, :, :],
                    in_=ag1_dst[hk * 4 * P:(hk + 1) * 4 * P, :].rearrange(
                        "(k p) (et n) -> p k et n", p=P, et=NE))
            for k in range(NCORES):
                for c in range(0, NL // P, 2):
                    mb = k * (NL // P) + c
                    pd = pdp.tile([P, 2, NL], F32, tag="pd", bufs=2, name="pd")
                    for h in range(2):
                        cs = slice((c + h) * P, (c + h + 1) * P)
                        nc.tensor.matmul(
                            pd[:, h, :], fhk[:, k, 0:2, cs], dg8[:, 0:2, :],
                            start=True, stop=False, perf_mode=DR)
                        nc.tensor.matmul(
                            pd[:, h, :], fhk[:, k, 2:4, cs], dg8[:, 2:4, :],
                            start=False, stop=True, perf_mode=DR)
                    nc.scalar.activation(
                        out=A0T[:, mb:mb + 2, :], in_=pd[:], func=AF.Sigmoid)
            pd_cm.__exit__(None, None, None)

        # ---- D2: r0 = A0@g0, q = A0@g1 (fp8 DoubleRow), reductions ----
        # et-outer with all 8 g-tiles resident: each et's PSUM accumulator
        # completes early so its reductions overlap the next et's matmuls.
        with tc.tile_pool(name="gb", bufs=1) as gbp, \
             tc.tile_pool(name="fin", bufs=1) as fin, \
             tc.tile_pool(name="pr", bufs=1, space="PSUM") as prp:
            def reduce_et(rp_, gT, et, sq_col, dve_col, want_sumq):
                scr = fin.tile([P, NL], F32, tag="scr", bufs=2, name="scr")
                nc.scalar.activation(
                    out=scr[:], in_=rp_[:], func=AF.Square,
                    accum_out=out_act[:, sq_col + et:sq_col + et + 1])
                scr2 = fin.tile([P, NL], F32, tag="scr2", bufs=2, name="scr2")
                nc.vector.scalar_tensor_tensor(
                    out=scr2[:], in0=rp_[:], scalar=1.0, in1=gT[:, et, :],
                    op0=ALU.mult, op1=ALU.mult,
                    accum_out=out_dve[:, dve_col + et:dve_col + et + 1])
                if want_sumq:
                    scr3 = fin.tile([P, NL], F32, tag="scr3", bufs=2,
                                    name="scr3")
                    nc.scalar.activation(
                        out=scr3[:], in_=rp_[:], func=AF.Copy,
                        accum_out=out_act[:, 4 + et:4 + et + 1])

            # q = A0@g1 first (its exchange lands first), then r0 = A0@g0
            for a_dst, gT, sq_col, dve_col, want_sumq in (
                    (ag2b_dst, g1T, 8, 4, True),
                    (ag2a_dst, g0T, 0, 0, False)):
                gpk = [gbp.tile([P, NE, E], FP8, tag=f"gpk{k}", bufs=2,
                                name=f"gpk{k}") for k in range(NCORES)]
                for k in range(NCORES):
                    nc.sync.dma_start(
                        out=gpk[k][:],
                        in_=a_dst[k * P:(k + 1) * P, :].rearrange(
                            "p (s e) -> p s e", s=NE))
                for et in range(NE):
                    rp_ = prp.tile([P, NL], F32, tag=f"r{et % 2}",
                                   bufs=2, name=f"r{et}")
                    es = slice(et * P, (et + 1) * P)
                    for k in range(NCORES):
                        for cp in range(2):
                            mp = k * 4 + 2 * cp
                            nc.tensor.matmul(
                                rp_[:], gpk[k][:, 2 * cp:2 * cp + 2, es],
                                A0T[:, mp:mp + 2, :],
                                start=(k == 0 and cp == 0),
                                stop=(k == NCORES - 1 and cp == 1),
                                perf_mode=DR)
                    reduce_et(rp_, gT, et, sq_col, dve_col, want_sumq)

            nc.sync.dma_start(out=T["out_a"][:], in_=out_act[:])
            nc.sync.dma_start(out=T["out_d"][:], in_=out_dve[:])


_PROGRAM = None


def _get_program():
    global _PROGRAM
    if _PROGRAM is None:
        _PROGRAM = build_program()
    return _PROGRAM


def _prep_w(w, gates=(0, 1, 2, 3)):
    """(4H, E_in) torch-layout weight -> bf16 lhsT tiles [p, kt, len(gates)*H]."""
    import ml_dtypes
    w = np.asarray(w, np.float32)
    h4 = w.shape[0]
    h = h4 // 4
    wt = np.concatenate([w[g * h:(g + 1) * h] for g in gates], 0).T
    e_in, cols = wt.shape
    return np.ascontiguousarray(
        wt.reshape(e_in // P, P, cols).transpose(1, 0, 2)
        .astype(ml_dtypes.bfloat16))


def _prep_b(b1, b2, gates=(0, 1, 2, 3)):
    s = np.asarray(b1, np.float32) + np.asarray(b2, np.float32)
    h = s.shape[0] // 4
    s = np.concatenate([s[g * h:(g + 1) * h] for g in gates], 0)
    return np.ascontiguousarray(s.reshape(-1, P).T)


def run_device(inputs, trace=False):
    """Shard inputs, run the 8-core SPMD program, return bass results."""
    nc = _get_program()
    emb = np.ascontiguousarray(np.asarray(inputs["embedding"], np.float32))
    iq = np.asarray(inputs["input"]).astype(np.int32).reshape(N, 1)
    ie = np.asarray(inputs["set_inputs"]).astype(np.int32)
    shared = {
        "emb": emb,
        "wgf": _prep_w(inputs["wih_gf"]), "wgr": _prep_w(inputs["wih_gr"]),
        "ugf": _prep_w(inputs["whh_gf"]), "ugr": _prep_w(inputs["whh_gr"]),
        "wf": _prep_w(inputs["wih_f"], gates=(0, 2, 3)),
        "bgf": _prep_b(inputs["bih_gf"], inputs["bhh_gf"]),
        "bgr": _prep_b(inputs["bih_gr"], inputs["bhh_gr"]),
        "bf": _prep_b(inputs["bih_f"], inputs["bhh_f"], gates=(0, 2, 3)),
    }
    in_maps = []
    for k in range(NCORES):
        sl = slice(k * NL, (k + 1) * NL)
        m = dict(shared)
        m["idx_x"] = np.ascontiguousarray(iq[sl])
        m["idx_e0"] = np.ascontiguousarray(ie[0, sl].reshape(NL, 1))
        m["idx_e1"] = np.ascontiguousarray(ie[1, sl].reshape(NL, 1))
        in_maps.append(m)
    res = bass_utils.run_bass_kernel_spmd(
        nc, in_maps, core_ids=list(range(NCORES)), trace=trace)
    return res


def kernel(**inputs):
    res = run_device(inputs)
    return host_tail(res, inputs)


def host_tail(res, inputs):
    # device partial layout:
    #   out_a cols: sr0(0:4) sqq(4:8)... see below; out_d cols: dot0, dot1q
    # column c of kind at base: value for e = et*128 + p
    sr0 = np.zeros(E, np.float64)
    sqq = np.zeros(E, np.float64)
    sumq = np.zeros(E, np.float64)
    sg0 = np.zeros(E, np.float64)
    sg1 = np.zeros(E, np.float64)
    t1 = np.zeros(E, np.float64)
    dot0 = np.zeros(E, np.float64)
    dot1q = np.zeros(E, np.float64)
    for r in res.results:
        a = np.asarray(r["out_a"], np.float64)   # (P, 24)
        d = np.asarray(r["out_d"], np.float64)   # (P, 8)
        for et in range(NE):
            sl = slice(et * P, (et + 1) * P)
            sr0[sl] += a[:, 0 + et]
            sumq[sl] += a[:, 4 + et]
            sqq[sl] += a[:, 8 + et]
            sg0[sl] += a[:, 12 + et]
            sg1[sl] += a[:, 16 + et]
            t1[sl] += a[:, 20 + et]
            dot0[sl] += d[:, et]
            dot1q[sl] += d[:, 4 + et]
    s1 = t1
    dot1 = s1 * t1 - dot1q
    sr1 = N * s1 ** 2 - 2.0 * s1 * sumq + sqq
    dot = np.stack([dot0, dot1])
    sr = np.stack([sr0, sr1])
    sg = np.stack([sg0, sg1])
    nr = np.maximum(np.sqrt(sr), EPS)
    ng = np.maximum(np.sqrt(sg), EPS)
    cos = dot / (nr * ng)
    kern = cos / np.exp(cos).sum()
    w_out = np.asarray(inputs["w_out"], np.float64)
    b_out = np.asarray(inputs["b_out"], np.float64)
    k2 = kern @ w_out.T + b_out
    s = k2.sum(axis=1)
    labels = np.asarray(inputs["set_labels"], np.float64)
    o = s[0] * labels[0] + s[1] * labels[1]
    o = np.exp(o - o.max())
    o /= o.sum()
    return o.astype(np.float32)
